# revision 15
# baseline (speedup 1.0000x reference)
"""Chamfer distance kernel for Trainium2 (Bass/Tile), SPMD over 8 NeuronCores.

Problem: source [8, 4096, 3], target [8, 4096, 3] float32.
  distance[b, n, m] = sum_c (source[b,n,c] - target[b,m,c])^2
  loss_src = mean_n min_m distance ; loss_dst = mean_m min_n distance
  returns (loss_src, loss_dst)

Sharding: batch b -> core b (data parallel; final means on host).

Grid-pruned exact KNN (IVF-style):
  Host prep (no pairwise point distances — bin geometry only):
    * Each direction (src->dst queries, dst->src queries) is tiled into 32
      compact tiles of 128 query points via recursive median bisection.
    * Reference points are binned on a uniform grid (cell h). For every
      query q, r(q) = dist(q, nearest occupied bin center) + half-diagonal
      upper-bounds its NN distance; every bin with mindist(q, bin) <= r(q)
      may hold the NN. The per-tile candidate set is the union of selected
      bins' members — provably a superset of every query's NN, so the
      tile row-min over candidates is the exact NN distance.
    * Candidate lists are chunked to <=512, slots sorted by width (desc),
      and per-slot widths maximized across the 8 cores (SPMD: one program).

  Device (per core, S ~ 65 slots):
    * slot = one bf16 matmul [16,128]x[16,w] -> PSUM [128,w] fp32 using the
      split-precision K=16 factorization (hi/lo bf16 pairs make the fp32
      products near-exact; see make_factors), then a row-min:
        - ACT path: ScalarE copies PSUM->SBUF bf16; DVE tensor_tensor_reduce
          folds halves (min) and accumulates the row-min in one op.
        - DVE path: DVE tensor_reduce min straight from PSUM (fp32, 1x).
      Paths are assigned greedily at build time to balance ScalarE vs DVE.
    * Output [128, S] fp32 row-minima; host combines split-tile slots with
      np.minimum and takes the final means in fp64.
"""

import os
import sys

import numpy as np

_TRN_REPO = "/opt/trn_rl_repo"
if _TRN_REPO not in sys.path and os.path.isdir(_TRN_REPO):
    sys.path.insert(0, _TRN_REPO)

from contextlib import ExitStack

import ml_dtypes
from scipy.spatial import cKDTree

import concourse.bacc as bacc
import concourse.mybir as mybir
import concourse.tile as tile

F32 = mybir.dt.float32
BF16 = mybir.dt.bfloat16
MIN = mybir.AluOpType.min
BF16NP = ml_dtypes.bfloat16
MIN_INIT = 1e30
K_AUG = 16  # rows of the split-precision augmented factors

B, N, M, C = 8, 4096, 4096, 3
N_CORES = 8
TILE = 128
WMAX = 512  # max slot width (one PSUM bank of fp32)
GRID_H = 0.07


# ---------------------------------------------------------------- host prep


def _split_bf16(x):
    x = np.asarray(x, np.float32)
    hi = x.astype(BF16NP)
    lo = (x - hi.astype(np.float32)).astype(BF16NP)
    return hi, lo


def make_factors(pts):
    """Per-point factor rows so that d(q, c) = sum_k QF[q, k] * CF[c, k].

    QF (query role):     [qh*3, qh*3, ql*3, ql*3, ah, al, 1, 1]
    CF (candidate role): [-2ch*3, -2cl*3, -2ch*3, -2cl*3, 1, 1, bh, bl]
    with x = xh + xl bf16 splits and a = ||q||^2, b = ||c||^2 split hi/lo.
    """
    p = np.asarray(pts, np.float32)
    ph, pl = _split_bf16(p)  # [n, 3] each
    nrm = (p.astype(np.float64) ** 2).sum(-1)
    nh, nl = _split_bf16(nrm)
    n = len(p)
    qf = np.zeros((n, K_AUG), BF16NP)
    qf[:, 0:3] = ph
    qf[:, 3:6] = ph
    qf[:, 6:9] = pl
    qf[:, 9:12] = pl
    qf[:, 12] = nh
    qf[:, 13] = nl
    qf[:, 14] = 1.0
    qf[:, 15] = 1.0
    cf = np.zeros((n, K_AUG), BF16NP)
    m2h = (-2.0 * ph.astype(np.float32)).astype(BF16NP)
    m2l = (-2.0 * pl.astype(np.float32)).astype(BF16NP)
    cf[:, 0:3] = m2h
    cf[:, 3:6] = m2l
    cf[:, 6:9] = m2h
    cf[:, 9:12] = m2l
    cf[:, 12] = 1.0
    cf[:, 13] = 1.0
    cf[:, 14] = nh
    cf[:, 15] = nl
    return qf, cf


def bisect_tiles(pts, tsize=TILE):
    """Permutation grouping points into compact boxes of `tsize`."""
    out = []

    def rec(ids):
        if len(ids) <= tsize:
            out.append(ids)
            return
        p = pts[ids]
        d = int(np.argmax(p.max(0) - p.min(0)))
        k = (len(ids) // 2 // tsize) * tsize or tsize
        part = np.argpartition(p[:, d], k)
        rec(ids[part[:k]])
        rec(ids[part[k:]])

    rec(np.arange(len(pts)))
    return np.concatenate(out)


def tile_candidates(q, t, h):
    """Exact-NN-complete candidate target ids per 128-query tile.

    Bin-granular: only grid geometry is used (no point-point distances).
    Returns (list_of_q_id_arrays, list_of_cand_id_arrays).
    """
    q = q.astype(np.float64)
    t = t.astype(np.float64)
    lo = np.minimum(q.min(0), t.min(0)) - 1e-9
    tb = np.floor((t - lo) / h).astype(np.int64)
    keys, inv = np.unique(tb, axis=0, return_inverse=True)
    order_m = np.argsort(inv, kind="stable")
    bin_start = np.searchsorted(inv[order_m], np.arange(len(keys) + 1))
    centers = lo + (keys + 0.5) * h
    hd = h * np.sqrt(3) / 2
    tree = cKDTree(centers)
    dc, _ = tree.query(q, k=1)
    r = dc + hd  # NN distance upper bound per query
    bin_lo = lo + keys * h
    bin_hi = bin_lo + h
    order = bisect_tiles(q)
    q_tiles, cand_tiles = [], []
    for ti in range(0, len(q), TILE):
        ids = order[ti : ti + TILE]
        s = q[ids]
        balls = tree.query_ball_point(s, r[ids] + hd)
        sel = np.zeros(len(keys), bool)
        for j, bl in enumerate(balls):
            bl = np.asarray(bl, dtype=np.int64)
            near = np.maximum(np.maximum(bin_lo[bl] - s[j], s[j] - bin_hi[bl]), 0)
            ok = (near**2).sum(-1) <= r[ids[j]] ** 2
            sel[bl[ok]] = True
        cand = np.concatenate(
            [order_m[bin_start[k] : bin_start[k + 1]] for k in np.nonzero(sel)[0]]
        )
        q_tiles.append(ids)
        cand_tiles.append(cand)
    return q_tiles, cand_tiles


def prep(source, target, h=GRID_H):
    """Build per-core slot tensors U [16, 128*S], V [16, sum(widths)].

    Returns (widths, u_all [B,16,128*S], v_all [B,16,Vtot], slot_maps) where
    slot_maps[core] = list of (direction, n_slots_for_tile) aligned with the
    tile traversal; real slots per core are the first len(map) entries after
    per-core sorting (we keep explicit per-core slot lists instead).
    """
    src = np.asarray(source, np.float32)
    tgt = np.asarray(target, np.float32)
    per_core = []  # core -> list of (dir, width_used, q_ids, cand_ids)
    for b in range(B):
        sf_q, sf_c = make_factors(src[b])
        tf_q, tf_c = make_factors(tgt[b])
        slots = []
        for d, (q, t, qf, cf) in enumerate(
            [
                (src[b], tgt[b], sf_q, tf_c),
                (tgt[b], src[b], tf_q, sf_c),
            ]
        ):
            q_tiles, cand_tiles = tile_candidates(q, t, h)
            for ids, cand in zip(q_tiles, cand_tiles):
                for c0 in range(0, len(cand), WMAX):
                    chunk = cand[c0 : c0 + WMAX]
                    slots.append((d, len(chunk), ids, chunk, qf, cf))
        slots.sort(key=lambda s: -s[1])
        per_core.append(slots)

    S = max(len(s) for s in per_core)
    widths = np.zeros(S, np.int64)
    for slots in per_core:
        for i, sl in enumerate(slots):
            widths[i] = max(widths[i], sl[1])
    # 128-multiples give long equal-width runs so slots pack into quads
    widths = np.minimum((widths + 127) // 128 * 128, WMAX)
    widths = np.maximum(widths, 128)
    widths = tuple(int(w) for w in widths)

    packs, uq, vq = slot_layout(widths)
    slot_pos = {}
    for s0, P, w, g, uoff, voff in packs:
        for j in range(P):
            slot_pos[s0 + j] = (g, uoff + j * TILE, voff + j * w)

    u_all = np.zeros((B, 3 * K_AUG, uq), BF16NP)
    v_all = np.zeros((B, 3 * K_AUG, vq), BF16NP)
    maps = []
    for b, slots in enumerate(per_core):
        core_map = []
        for i in range(S):
            w = widths[i]
            g, uo, vo = slot_pos[i]
            r = K_AUG * g
            if i < len(slots):
                d, wu, ids, cand, qf, cf = slots[i]
                pad = np.concatenate([cand, np.repeat(cand[:1], w - len(cand))])
                u_all[b, r : r + K_AUG, uo : uo + TILE] = qf[ids].T
                v_all[b, r : r + K_AUG, vo : vo + w] = cf[pad].T
                core_map.append((d, ids))
            else:
                core_map.append(None)
        maps.append(core_map)
    return widths, u_all, v_all, maps


# ------------------------------------------------------------- device build


def slot_layout(widths):
    """Packs of up to 4 equal-width slots, round-robin over 3 PE quadrants.

    Returns (packs, uq, vq): packs = [(s0, P, w, quadrant, uoff, voff)];
    uq/vq = per-quadrant column capacity (max across quadrants).
    """
    S = len(widths)
    raw = []
    s = 0
    while s < S:
        p = 1
        while p < 4 and s + p < S and widths[s + p] == widths[s]:
            p += 1
        raw.append((s, p, widths[s]))
        s += p
    packs = []
    ucols = [0, 0, 0]
    vcols = [0, 0, 0]
    for s0, P, w in raw:
        g = min(range(3), key=lambda q: vcols[q])
        packs.append((s0, P, w, g, ucols[g], vcols[g]))
        ucols[g] += P * TILE
        vcols[g] += P * w
    return packs, max(ucols), max(vcols)


def build_knn_nc(widths, do_compile=True):
    S = len(widths)
    packs, uq, vq = slot_layout(widths)

    nc = bacc.Bacc("TRN2", target_bir_lowering=False, debug=False)
    u_d = nc.dram_tensor("u_in", [3 * K_AUG, uq], BF16, kind="ExternalInput").ap()
    v_d = nc.dram_tensor("v_in", [3 * K_AUG, vq], BF16, kind="ExternalInput").ap()
    out_d = nc.dram_tensor("out", [TILE, S], F32, kind="ExternalOutput").ap()

    with tile.TileContext(nc) as tc, ExitStack() as ctx:
        const_pool = ctx.enter_context(tc.tile_pool(name="const", bufs=1))
        psum_pool = ctx.enter_context(tc.tile_pool(name="psum", bufs=2, space="PSUM"))
        dd_pool = ctx.enter_context(tc.tile_pool(name="dd", bufs=3))
        f1_pool = ctx.enter_context(tc.tile_pool(name="f1", bufs=2))
        f2_pool = ctx.enter_context(tc.tile_pool(name="f2", bufs=2))

        # quadrant q data sits on SBUF partitions 32q..32q+15 so the three
        # input DMAs write 48 partitions concurrently (3x bandwidth)
        ut = const_pool.tile([TILE, uq], BF16, tag="u")
        vt = const_pool.tile([TILE, vq], BF16, tag="v")
        for g in range(3):
            nc.scalar.dma_start(ut[32 * g : 32 * g + K_AUG, :], u_d[K_AUG * g : K_AUG * (g + 1), :])
            nc.sync.dma_start(vt[32 * g : 32 * g + K_AUG, :], v_d[K_AUG * g : K_AUG * (g + 1), :])

        outs = const_pool.tile([TILE, S], F32, tag="outs")

        # greedy 3-engine balance (ns models; ACT 1.2GHz, DVE 0.96GHz, GPS)
        act_busy = dve_busy = gps_busy = 0.0
        half_s = packs[(len(packs) // 2)][0]
        for pi, (s0, P, w, g, uoff, voff) in enumerate(packs):
            ps = psum_pool.tile([TILE, 4, WMAX], F32, tag="ps")
            for j in range(P):
                nc.tensor.matmul(
                    ps[:, j, :w],
                    ut[32 * g : 32 * g + K_AUG, uoff + j * TILE : uoff + (j + 1) * TILE],
                    vt[32 * g : 32 * g + K_AUG, voff + j * w : voff + (j + 1) * w],
                    start=True,
                    stop=True,
                )
            n = P * w
            act_copy = (352 + n) / 1.2
            dve_foldall = (174 + 0.625 * n) / 0.96  # fold/2 + fold/4 + reduce
            dve_foldtail = (116 + 0.375 * n) / 0.96  # fold/4 + reduce (GPS did fold1)
            gps_fold = 300 + 2.2 * n / 2
            dve_direct = (120 + n) / 0.96
            last = pi == len(packs) - 1
            cost = {
                "A": max(act_busy + act_copy, dve_busy + dve_foldall, gps_busy),
                "C": max(act_busy, dve_busy + dve_direct, gps_busy),
            }
            path = "C" if last else min(cost, key=cost.get)
            if path == "C":
                dve_busy += dve_direct
                nc.vector.tensor_reduce(
                    outs[:, s0 : s0 + P], ps[:, :P, :w], axis=mybir.AxisListType.X, op=MIN
                )
            else:
                act_busy += act_copy
                dd = dd_pool.tile([TILE, 4, WMAX], BF16, tag="dd")
                nc.scalar.copy(dd[:, :P, :w], ps[:, :P, :w])
                f1 = f1_pool.tile([TILE, 4, WMAX // 2], BF16, tag="f1")
                eng = nc.gpsimd if path == "B" else nc.vector
                if path == "B":
                    gps_busy += gps_fold
                else:
                    dve_busy += dve_foldall - dve_foldtail
                eng.tensor_tensor(
                    f1[:, :P, : w // 2], dd[:, :P, : w // 2], dd[:, :P, w // 2 : w], MIN
                )
                dve_busy += dve_foldtail
                f2 = f2_pool.tile([TILE, 4, WMAX // 4], BF16, tag="f2")
                nc.vector.tensor_tensor(
                    f2[:, :P, : w // 4], f1[:, :P, : w // 4], f1[:, :P, w // 4 : w // 2], MIN
                )
                nc.vector.tensor_reduce(
                    outs[:, s0 : s0 + P], f2[:, :P, : w // 4], axis=mybir.AxisListType.X, op=MIN
                )
            if s0 + P == half_s:
                nc.sync.dma_start(out_d[:, :half_s], outs[:, :half_s])

        nc.sync.dma_start(out_d[:, half_s:], outs[:, half_s:])

    if do_compile:
        nc.compile()
    return nc


_NC_CACHE = {}


def _get_nc(widths):
    if widths not in _NC_CACHE:
        _NC_CACHE[widths] = build_knn_nc(widths)
    return _NC_CACHE[widths]


def run_device(widths, u_all, v_all, trace: bool = False):
    from concourse.bass_utils import run_bass_kernel_spmd

    nc = _get_nc(widths)
    in_maps = [{"u_in": u_all[c], "v_in": v_all[c]} for c in range(N_CORES)]
    res = run_bass_kernel_spmd(nc, in_maps, list(range(N_CORES)), trace=trace)
    return res


def postprocess(res, maps):
    """Combine slot row-minima into the two mean losses (fp64)."""
    sums = np.zeros(2, np.float64)
    counts = np.zeros(2, np.int64)
    for c in range(N_CORES):
        out = np.asarray(res.results[c]["out"], np.float64)  # [128, S]
        # split tiles: same (dir, ids) may appear in multiple slots
        seen = {}
        for s, m in enumerate(maps[c]):
            if m is None:
                continue
            d, ids = m
            key = (d, ids[0])
            if key in seen:
                seen[key] = np.minimum(seen[key], out[:, s])
            else:
                seen[key] = out[:, s]
        for (d, _), vals in seen.items():
            sums[d] += vals.sum()
            counts[d] += len(vals)
    assert counts[0] == B * N and counts[1] == B * M, (counts, B * N)
    return np.float32(sums[0] / counts[0]), np.float32(sums[1] / counts[1])


def kernel(source: np.ndarray, target: np.ndarray):
    widths, u_all, v_all, maps = prep(source, target)
    res = run_device(widths, u_all, v_all)
    return postprocess(res, maps)


# revision 17
# speedup vs baseline: 1.2393x; 1.2393x over previous
"""Chamfer distance kernel for Trainium2 (Bass/Tile), SPMD over 8 NeuronCores.

Problem: source [8, 4096, 3], target [8, 4096, 3] float32.
  distance[b, n, m] = sum_c (source[b,n,c] - target[b,m,c])^2
  loss_src = mean_n min_m distance ; loss_dst = mean_m min_n distance
  returns (loss_src, loss_dst)

Sharding: batch b -> core b (data parallel; final means on host).

Grid-pruned exact KNN (IVF-style):
  Host prep (no pairwise point distances — bin geometry only):
    * Each direction (src->dst queries, dst->src queries) is tiled into 32
      compact tiles of 128 query points via recursive median bisection.
    * Reference points are binned on a uniform grid (cell h). For every
      query q, r(q) = dist(q, nearest occupied bin center) + half-diagonal
      upper-bounds its NN distance; every bin with mindist(q, bin) <= r(q)
      may hold the NN. The per-tile candidate set is the union of selected
      bins' members — provably a superset of every query's NN, so the
      tile row-min over candidates is the exact NN distance.
    * Candidate lists are chunked to <=512, slots sorted by width (desc),
      and per-slot widths maximized across the 8 cores (SPMD: one program).

  Device (per core, S ~ 65 slots):
    * slot = one bf16 matmul [16,128]x[16,w] -> PSUM [128,w] fp32 using the
      split-precision K=16 factorization (hi/lo bf16 pairs make the fp32
      products near-exact; see make_factors), then a row-min:
        - ACT path: ScalarE copies PSUM->SBUF bf16; DVE tensor_tensor_reduce
          folds halves (min) and accumulates the row-min in one op.
        - DVE path: DVE tensor_reduce min straight from PSUM (fp32, 1x).
      Paths are assigned greedily at build time to balance ScalarE vs DVE.
    * Output [128, S] fp32 row-minima; host combines split-tile slots with
      np.minimum and takes the final means in fp64.
"""

import os
import sys

import numpy as np

_TRN_REPO = "/opt/trn_rl_repo"
if _TRN_REPO not in sys.path and os.path.isdir(_TRN_REPO):
    sys.path.insert(0, _TRN_REPO)

from contextlib import ExitStack

import ml_dtypes
from scipy.spatial import cKDTree

import concourse.bacc as bacc
import concourse.mybir as mybir
import concourse.tile as tile

F32 = mybir.dt.float32
BF16 = mybir.dt.bfloat16
MIN = mybir.AluOpType.min
BF16NP = ml_dtypes.bfloat16
MIN_INIT = 1e30
K_AUG = 16  # rows of the split-precision augmented factors

B, N, M, C = 8, 4096, 4096, 3
N_CORES = 8
TILE = 128
WMAX = 512  # max slot width (one PSUM bank of fp32)
GRID_H = 0.018


# ---------------------------------------------------------------- host prep


def _split_bf16(x):
    x = np.asarray(x, np.float32)
    hi = x.astype(BF16NP)
    lo = (x - hi.astype(np.float32)).astype(BF16NP)
    return hi, lo


def make_factors(pts):
    """Per-point factor rows so that d(q, c) = sum_k QF[q, k] * CF[c, k].

    QF (query role):     [qh*3, qh*3, ql*3, ql*3, ah, al, 1, 1]
    CF (candidate role): [-2ch*3, -2cl*3, -2ch*3, -2cl*3, 1, 1, bh, bl]
    with x = xh + xl bf16 splits and a = ||q||^2, b = ||c||^2 split hi/lo.
    """
    p = np.asarray(pts, np.float32)
    ph, pl = _split_bf16(p)  # [n, 3] each
    nrm = (p.astype(np.float64) ** 2).sum(-1)
    nh, nl = _split_bf16(nrm)
    n = len(p)
    qf = np.zeros((n, K_AUG), BF16NP)
    qf[:, 0:3] = ph
    qf[:, 3:6] = ph
    qf[:, 6:9] = pl
    qf[:, 9:12] = pl
    qf[:, 12] = nh
    qf[:, 13] = nl
    qf[:, 14] = 1.0
    qf[:, 15] = 1.0
    cf = np.zeros((n, K_AUG), BF16NP)
    m2h = (-2.0 * ph.astype(np.float32)).astype(BF16NP)
    m2l = (-2.0 * pl.astype(np.float32)).astype(BF16NP)
    cf[:, 0:3] = m2h
    cf[:, 3:6] = m2l
    cf[:, 6:9] = m2h
    cf[:, 9:12] = m2l
    cf[:, 12] = 1.0
    cf[:, 13] = 1.0
    cf[:, 14] = nh
    cf[:, 15] = nl
    return qf, cf


def bisect_tiles(pts, tsize=TILE):
    """Permutation grouping points into compact boxes of `tsize`."""
    out = []

    def rec(ids):
        if len(ids) <= tsize:
            out.append(ids)
            return
        p = pts[ids]
        d = int(np.argmax(p.max(0) - p.min(0)))
        k = (len(ids) // 2 // tsize) * tsize or tsize
        part = np.argpartition(p[:, d], k)
        rec(ids[part[:k]])
        rec(ids[part[k:]])

    rec(np.arange(len(pts)))
    return np.concatenate(out)


def tile_candidates(q, t, h):
    """Exact-NN-complete candidate target ids per 128-query tile.

    Bin-granular: only grid geometry is used (no point-point distances).
    Returns (list_of_q_id_arrays, list_of_cand_id_arrays).
    """
    q = q.astype(np.float64)
    t = t.astype(np.float64)
    lo = np.minimum(q.min(0), t.min(0)) - 1e-9
    tb = np.floor((t - lo) / h).astype(np.int64)
    keys, inv = np.unique(tb, axis=0, return_inverse=True)
    order_m = np.argsort(inv, kind="stable")
    bin_start = np.searchsorted(inv[order_m], np.arange(len(keys) + 1))
    centers = lo + (keys + 0.5) * h
    hd = h * np.sqrt(3) / 2
    tree = cKDTree(centers)
    bin_lo_all = lo + keys * h
    bin_hi_all = bin_lo_all + h
    kq = min(8, len(keys))
    _, ci = tree.query(q, k=kq)
    ci = ci.reshape(len(q), kq)
    far = np.maximum(
        np.abs(q[:, None, :] - bin_lo_all[ci]), np.abs(q[:, None, :] - bin_hi_all[ci])
    )
    r = np.sqrt((far**2).sum(-1).min(1))  # NN distance upper bound per query
    bin_lo = lo + keys * h
    bin_hi = bin_lo + h
    order = bisect_tiles(q)
    q_tiles, cand_tiles = [], []
    for ti in range(0, len(q), TILE):
        ids = order[ti : ti + TILE]
        s = q[ids]
        balls = tree.query_ball_point(s, r[ids] + hd)
        sel = np.zeros(len(keys), bool)
        for j, bl in enumerate(balls):
            bl = np.asarray(bl, dtype=np.int64)
            near = np.maximum(np.maximum(bin_lo[bl] - s[j], s[j] - bin_hi[bl]), 0)
            ok = (near**2).sum(-1) <= r[ids[j]] ** 2
            sel[bl[ok]] = True
        cand = np.concatenate(
            [order_m[bin_start[k] : bin_start[k + 1]] for k in np.nonzero(sel)[0]]
        )
        q_tiles.append(ids)
        cand_tiles.append(cand)
    return q_tiles, cand_tiles


def prep(source, target, h=GRID_H):
    """Build per-core slot tensors U [16, 128*S], V [16, sum(widths)].

    Returns (widths, u_all [B,16,128*S], v_all [B,16,Vtot], slot_maps) where
    slot_maps[core] = list of (direction, n_slots_for_tile) aligned with the
    tile traversal; real slots per core are the first len(map) entries after
    per-core sorting (we keep explicit per-core slot lists instead).
    """
    src = np.asarray(source, np.float32)
    tgt = np.asarray(target, np.float32)
    per_core = []  # core -> list of (dir, width_used, q_ids, cand_ids)
    for b in range(B):
        sf_q, sf_c = make_factors(src[b])
        tf_q, tf_c = make_factors(tgt[b])
        slots = []
        for d, (q, t, qf, cf) in enumerate(
            [
                (src[b], tgt[b], sf_q, tf_c),
                (tgt[b], src[b], tf_q, sf_c),
            ]
        ):
            q_tiles, cand_tiles = tile_candidates(q, t, h)
            for ids, cand in zip(q_tiles, cand_tiles):
                for c0 in range(0, len(cand), WMAX):
                    chunk = cand[c0 : c0 + WMAX]
                    slots.append((d, len(chunk), ids, chunk, qf, cf))
        slots.sort(key=lambda s: -s[1])
        per_core.append(slots)

    S = max(len(s) for s in per_core)
    widths = np.zeros(S, np.int64)
    for slots in per_core:
        for i, sl in enumerate(slots):
            widths[i] = max(widths[i], sl[1])
    widths = np.minimum((widths + 63) // 64 * 64, WMAX)
    widths = np.maximum(widths, 64)
    # pad each quad-pack to its max width so packs stay equal-width (one
    # ACT copy / DVE fold+reduce per pack) without blanket 128-rounding
    for i in range(0, S, 4):
        widths[i : i + 4] = widths[i]
    widths = tuple(int(w) for w in widths)

    packs, uq, vq = slot_layout(widths)
    slot_pos = {}
    for s0, P, w, g, uoff, voff in packs:
        for j in range(P):
            slot_pos[s0 + j] = (g, uoff + j * TILE, voff + j * w)

    u_all = np.zeros((B, 3 * K_AUG, uq), BF16NP)
    v_all = np.zeros((B, 3 * K_AUG, vq), BF16NP)
    maps = []
    for b, slots in enumerate(per_core):
        core_map = []
        for i in range(S):
            w = widths[i]
            g, uo, vo = slot_pos[i]
            r = K_AUG * g
            if i < len(slots):
                d, wu, ids, cand, qf, cf = slots[i]
                pad = np.concatenate([cand, np.repeat(cand[:1], w - len(cand))])
                u_all[b, r : r + K_AUG, uo : uo + TILE] = qf[ids].T
                v_all[b, r : r + K_AUG, vo : vo + w] = cf[pad].T
                core_map.append((d, ids))
            else:
                core_map.append(None)
        maps.append(core_map)
    return widths, u_all, v_all, maps


# ------------------------------------------------------------- device build


def slot_layout(widths):
    """Packs of up to 4 equal-width slots, round-robin over 3 PE quadrants.

    Returns (packs, uq, vq): packs = [(s0, P, w, quadrant, uoff, voff)];
    uq/vq = per-quadrant column capacity (max across quadrants).
    """
    S = len(widths)
    raw = []
    s = 0
    while s < S:
        p = 1
        while p < 4 and s + p < S and widths[s + p] == widths[s]:
            p += 1
        raw.append((s, p, widths[s]))
        s += p
    packs = []
    ucols = [0, 0, 0]
    vcols = [0, 0, 0]
    for s0, P, w in raw:
        g = min(range(3), key=lambda q: vcols[q])
        packs.append((s0, P, w, g, ucols[g], vcols[g]))
        ucols[g] += P * TILE
        vcols[g] += P * w
    return packs, max(ucols), max(vcols)


def build_knn_nc(widths, do_compile=True):
    S = len(widths)
    packs, uq, vq = slot_layout(widths)

    nc = bacc.Bacc("TRN2", target_bir_lowering=False, debug=False)
    u_d = nc.dram_tensor("u_in", [3 * K_AUG, uq], BF16, kind="ExternalInput").ap()
    v_d = nc.dram_tensor("v_in", [3 * K_AUG, vq], BF16, kind="ExternalInput").ap()
    out_d = nc.dram_tensor("out", [TILE, S], F32, kind="ExternalOutput").ap()

    with tile.TileContext(nc) as tc, ExitStack() as ctx:
        const_pool = ctx.enter_context(tc.tile_pool(name="const", bufs=1))
        psum_pool = ctx.enter_context(tc.tile_pool(name="psum", bufs=2, space="PSUM"))
        dd_pool = ctx.enter_context(tc.tile_pool(name="dd", bufs=3))
        f1_pool = ctx.enter_context(tc.tile_pool(name="f1", bufs=2))
        f2_pool = ctx.enter_context(tc.tile_pool(name="f2", bufs=2))

        # quadrant q data sits on SBUF partitions 32q..32q+15 so the three
        # input DMAs write 48 partitions concurrently (3x bandwidth)
        ut = const_pool.tile([TILE, uq], BF16, tag="u")
        vt = const_pool.tile([TILE, vq], BF16, tag="v")
        for g in range(3):
            nc.scalar.dma_start(ut[32 * g : 32 * g + K_AUG, :], u_d[K_AUG * g : K_AUG * (g + 1), :])
            nc.sync.dma_start(vt[32 * g : 32 * g + K_AUG, :], v_d[K_AUG * g : K_AUG * (g + 1), :])

        outs = const_pool.tile([TILE, S], F32, tag="outs")

        # greedy 3-engine balance (ns models; ACT 1.2GHz, DVE 0.96GHz, GPS)
        act_busy = dve_busy = gps_busy = 0.0
        half_s = packs[(len(packs) // 2)][0]
        for pi, (s0, P, w, g, uoff, voff) in enumerate(packs):
            ps = psum_pool.tile([TILE, 4, WMAX], F32, tag="ps")
            for j in range(P):
                nc.tensor.matmul(
                    ps[:, j, :w],
                    ut[32 * g : 32 * g + K_AUG, uoff + j * TILE : uoff + (j + 1) * TILE],
                    vt[32 * g : 32 * g + K_AUG, voff + j * w : voff + (j + 1) * w],
                    start=True,
                    stop=True,
                )
            n = P * w
            act_copy = (352 + n) / 1.2
            dve_foldall = (174 + 0.625 * n) / 0.96  # fold/2 + fold/4 + reduce
            dve_foldtail = (116 + 0.375 * n) / 0.96  # fold/4 + reduce (GPS did fold1)
            gps_fold = 300 + 2.2 * n / 2
            dve_direct = (120 + n) / 0.96
            last = pi == len(packs) - 1
            cost = {
                "A": max(act_busy + act_copy, dve_busy + dve_foldall, gps_busy),
                "C": max(act_busy, dve_busy + dve_direct, gps_busy),
            }
            path = "C" if last else min(cost, key=cost.get)
            if path == "C":
                dve_busy += dve_direct
                nc.vector.tensor_reduce(
                    outs[:, s0 : s0 + P], ps[:, :P, :w], axis=mybir.AxisListType.X, op=MIN
                )
            else:
                act_busy += act_copy
                dd = dd_pool.tile([TILE, 4, WMAX], BF16, tag="dd")
                nc.scalar.copy(dd[:, :P, :w], ps[:, :P, :w])
                f1 = f1_pool.tile([TILE, 4, WMAX // 2], BF16, tag="f1")
                eng = nc.gpsimd if path == "B" else nc.vector
                if path == "B":
                    gps_busy += gps_fold
                else:
                    dve_busy += dve_foldall - dve_foldtail
                eng.tensor_tensor(
                    f1[:, :P, : w // 2], dd[:, :P, : w // 2], dd[:, :P, w // 2 : w], MIN
                )
                dve_busy += dve_foldtail
                f2 = f2_pool.tile([TILE, 4, WMAX // 4], BF16, tag="f2")
                nc.vector.tensor_tensor(
                    f2[:, :P, : w // 4], f1[:, :P, : w // 4], f1[:, :P, w // 4 : w // 2], MIN
                )
                nc.vector.tensor_reduce(
                    outs[:, s0 : s0 + P], f2[:, :P, : w // 4], axis=mybir.AxisListType.X, op=MIN
                )
            if s0 + P == half_s:
                nc.sync.dma_start(out_d[:, :half_s], outs[:, :half_s])

        nc.sync.dma_start(out_d[:, half_s:], outs[:, half_s:])

    if do_compile:
        nc.compile()
    return nc


_NC_CACHE = {}


def _get_nc(widths):
    if widths not in _NC_CACHE:
        _NC_CACHE[widths] = build_knn_nc(widths)
    return _NC_CACHE[widths]


def run_device(widths, u_all, v_all, trace: bool = False):
    from concourse.bass_utils import run_bass_kernel_spmd

    nc = _get_nc(widths)
    in_maps = [{"u_in": u_all[c], "v_in": v_all[c]} for c in range(N_CORES)]
    res = run_bass_kernel_spmd(nc, in_maps, list(range(N_CORES)), trace=trace)
    return res


def postprocess(res, maps):
    """Combine slot row-minima into the two mean losses (fp64)."""
    sums = np.zeros(2, np.float64)
    counts = np.zeros(2, np.int64)
    for c in range(N_CORES):
        out = np.asarray(res.results[c]["out"], np.float64)  # [128, S]
        # split tiles: same (dir, ids) may appear in multiple slots
        seen = {}
        for s, m in enumerate(maps[c]):
            if m is None:
                continue
            d, ids = m
            key = (d, ids[0])
            if key in seen:
                seen[key] = np.minimum(seen[key], out[:, s])
            else:
                seen[key] = out[:, s]
        for (d, _), vals in seen.items():
            sums[d] += vals.sum()
            counts[d] += len(vals)
    assert counts[0] == B * N and counts[1] == B * M, (counts, B * N)
    return np.float32(sums[0] / counts[0]), np.float32(sums[1] / counts[1])


def kernel(source: np.ndarray, target: np.ndarray):
    widths, u_all, v_all, maps = prep(source, target)
    res = run_device(widths, u_all, v_all)
    return postprocess(res, maps)


# revision 18
# speedup vs baseline: 1.3368x; 1.0787x over previous
"""Chamfer distance kernel for Trainium2 (Bass/Tile), SPMD over 8 NeuronCores.

Problem: source [8, 4096, 3], target [8, 4096, 3] float32.
  distance[b, n, m] = sum_c (source[b,n,c] - target[b,m,c])^2
  loss_src = mean_n min_m distance ; loss_dst = mean_m min_n distance
  returns (loss_src, loss_dst)

Sharding: batch b -> core b (data parallel; final means on host).

Grid-pruned exact KNN (IVF-style):
  Host prep (no pairwise point distances — bin geometry only):
    * Each direction (src->dst queries, dst->src queries) is tiled into 32
      compact tiles of 128 query points via recursive median bisection.
    * Reference points are binned on a uniform grid (cell h). For every
      query q, r(q) = dist(q, nearest occupied bin center) + half-diagonal
      upper-bounds its NN distance; every bin with mindist(q, bin) <= r(q)
      may hold the NN. The per-tile candidate set is the union of selected
      bins' members — provably a superset of every query's NN, so the
      tile row-min over candidates is the exact NN distance.
    * Candidate lists are chunked to <=512, slots sorted by width (desc),
      and per-slot widths maximized across the 8 cores (SPMD: one program).

  Device (per core, S ~ 65 slots):
    * slot = one bf16 matmul [16,128]x[16,w] -> PSUM [128,w] fp32 using the
      split-precision K=16 factorization (hi/lo bf16 pairs make the fp32
      products near-exact; see make_factors), then a row-min:
        - ACT path: ScalarE copies PSUM->SBUF bf16; DVE tensor_tensor_reduce
          folds halves (min) and accumulates the row-min in one op.
        - DVE path: DVE tensor_reduce min straight from PSUM (fp32, 1x).
      Paths are assigned greedily at build time to balance ScalarE vs DVE.
    * Output [128, S] fp32 row-minima; host combines split-tile slots with
      np.minimum and takes the final means in fp64.
"""

import os
import sys

import numpy as np

_TRN_REPO = "/opt/trn_rl_repo"
if _TRN_REPO not in sys.path and os.path.isdir(_TRN_REPO):
    sys.path.insert(0, _TRN_REPO)

from contextlib import ExitStack

import ml_dtypes
from scipy.spatial import cKDTree

import concourse.bacc as bacc
import concourse.mybir as mybir
import concourse.tile as tile

F32 = mybir.dt.float32
BF16 = mybir.dt.bfloat16
MIN = mybir.AluOpType.min
BF16NP = ml_dtypes.bfloat16
MIN_INIT = 1e30
K_AUG = 16  # rows of the split-precision augmented factors

B, N, M, C = 8, 4096, 4096, 3
N_CORES = 8
TILE = 128
WMAX = 512  # max slot width (one PSUM bank of fp32)
GRID_H = 0.018


# ---------------------------------------------------------------- host prep


def _split_bf16(x):
    x = np.asarray(x, np.float32)
    hi = x.astype(BF16NP)
    lo = (x - hi.astype(np.float32)).astype(BF16NP)
    return hi, lo


def make_factors(pts):
    """Per-point factor rows so that d(q, c) = sum_k QF[q, k] * CF[c, k].

    QF (query role):     [qh*3, qh*3, ql*3, ql*3, ah, al, 1, 1]
    CF (candidate role): [-2ch*3, -2cl*3, -2ch*3, -2cl*3, 1, 1, bh, bl]
    with x = xh + xl bf16 splits and a = ||q||^2, b = ||c||^2 split hi/lo.
    """
    p = np.asarray(pts, np.float32)
    ph, pl = _split_bf16(p)  # [n, 3] each
    nrm = (p.astype(np.float64) ** 2).sum(-1)
    nh, nl = _split_bf16(nrm)
    n = len(p)
    qf = np.zeros((n, K_AUG), BF16NP)
    qf[:, 0:3] = ph
    qf[:, 3:6] = ph
    qf[:, 6:9] = pl
    qf[:, 9:12] = pl
    qf[:, 12] = nh
    qf[:, 13] = nl
    qf[:, 14] = 1.0
    qf[:, 15] = 1.0
    cf = np.zeros((n, K_AUG), BF16NP)
    m2h = (-2.0 * ph.astype(np.float32)).astype(BF16NP)
    m2l = (-2.0 * pl.astype(np.float32)).astype(BF16NP)
    cf[:, 0:3] = m2h
    cf[:, 3:6] = m2l
    cf[:, 6:9] = m2h
    cf[:, 9:12] = m2l
    cf[:, 12] = 1.0
    cf[:, 13] = 1.0
    cf[:, 14] = nh
    cf[:, 15] = nl
    return qf, cf


def bisect_tiles(pts, tsize=TILE):
    """Permutation grouping points into compact boxes of `tsize`."""
    out = []

    def rec(ids):
        if len(ids) <= tsize:
            out.append(ids)
            return
        p = pts[ids]
        d = int(np.argmax(p.max(0) - p.min(0)))
        k = (len(ids) // 2 // tsize) * tsize or tsize
        part = np.argpartition(p[:, d], k)
        rec(ids[part[:k]])
        rec(ids[part[k:]])

    rec(np.arange(len(pts)))
    return np.concatenate(out)


def tile_candidates(q, t, h):
    """Exact-NN-complete candidate target ids per 128-query tile.

    Bin-granular: only grid geometry is used (no point-point distances).
    Returns (list_of_q_id_arrays, list_of_cand_id_arrays).
    """
    q = q.astype(np.float64)
    t = t.astype(np.float64)
    lo = np.minimum(q.min(0), t.min(0)) - 1e-9
    tb = np.floor((t - lo) / h).astype(np.int64)
    keys, inv = np.unique(tb, axis=0, return_inverse=True)
    order_m = np.argsort(inv, kind="stable")
    bin_start = np.searchsorted(inv[order_m], np.arange(len(keys) + 1))
    centers = lo + (keys + 0.5) * h
    hd = h * np.sqrt(3) / 2
    tree = cKDTree(centers)
    bin_lo_all = lo + keys * h
    bin_hi_all = bin_lo_all + h
    kq = min(8, len(keys))
    _, ci = tree.query(q, k=kq)
    ci = ci.reshape(len(q), kq)
    far = np.maximum(
        np.abs(q[:, None, :] - bin_lo_all[ci]), np.abs(q[:, None, :] - bin_hi_all[ci])
    )
    r = np.sqrt((far**2).sum(-1).min(1))  # NN distance upper bound per query
    bin_lo = lo + keys * h
    bin_hi = bin_lo + h
    order = bisect_tiles(q)
    q_tiles, cand_tiles = [], []
    for ti in range(0, len(q), TILE):
        ids = order[ti : ti + TILE]
        s = q[ids]
        balls = tree.query_ball_point(s, r[ids] + hd)
        sel = np.zeros(len(keys), bool)
        for j, bl in enumerate(balls):
            bl = np.asarray(bl, dtype=np.int64)
            near = np.maximum(np.maximum(bin_lo[bl] - s[j], s[j] - bin_hi[bl]), 0)
            ok = (near**2).sum(-1) <= r[ids[j]] ** 2
            sel[bl[ok]] = True
        cand = np.concatenate(
            [order_m[bin_start[k] : bin_start[k + 1]] for k in np.nonzero(sel)[0]]
        )
        q_tiles.append(ids)
        cand_tiles.append(cand)
    return q_tiles, cand_tiles


def prep(source, target, h=GRID_H):
    """Build per-core slot tensors U [16, 128*S], V [16, sum(widths)].

    Returns (widths, u_all [B,16,128*S], v_all [B,16,Vtot], slot_maps) where
    slot_maps[core] = list of (direction, n_slots_for_tile) aligned with the
    tile traversal; real slots per core are the first len(map) entries after
    per-core sorting (we keep explicit per-core slot lists instead).
    """
    src = np.asarray(source, np.float32)
    tgt = np.asarray(target, np.float32)
    per_core = []  # core -> list of (dir, width_used, q_ids, cand_ids)
    for b in range(B):
        sf_q, sf_c = make_factors(src[b])
        tf_q, tf_c = make_factors(tgt[b])
        slots = []
        for d, (q, t, qf, cf) in enumerate(
            [
                (src[b], tgt[b], sf_q, tf_c),
                (tgt[b], src[b], tf_q, sf_c),
            ]
        ):
            q_tiles, cand_tiles = tile_candidates(q, t, h)
            for ids, cand in zip(q_tiles, cand_tiles):
                for c0 in range(0, len(cand), WMAX):
                    chunk = cand[c0 : c0 + WMAX]
                    slots.append((d, len(chunk), ids, chunk, qf, cf))
        slots.sort(key=lambda s: -s[1])
        per_core.append(slots)

    S = max(len(s) for s in per_core)
    widths = np.zeros(S, np.int64)
    for slots in per_core:
        for i, sl in enumerate(slots):
            widths[i] = max(widths[i], sl[1])
    widths = np.minimum((widths + 63) // 64 * 64, WMAX)
    widths = np.maximum(widths, 64)
    # pad each quad-pack to its max width so packs stay equal-width (one
    # ACT copy / DVE fold+reduce per pack) without blanket 128-rounding
    for i in range(0, S, 4):
        widths[i : i + 4] = widths[i]
    widths = tuple(int(w) for w in widths)

    packs, uq, vq = slot_layout(widths)
    slot_pos = {}
    for s0, P, w, g, uoff, voff in packs:
        for j in range(P):
            slot_pos[s0 + j] = (g, uoff + j * TILE, voff + j * w)

    u_all = np.zeros((B, 3 * K_AUG, uq), BF16NP)
    v_all = np.zeros((B, 3 * K_AUG, vq), BF16NP)
    maps = []
    for b, slots in enumerate(per_core):
        core_map = []
        for i in range(S):
            w = widths[i]
            g, uo, vo = slot_pos[i]
            r = K_AUG * g
            if i < len(slots):
                d, wu, ids, cand, qf, cf = slots[i]
                pad = np.concatenate([cand, np.repeat(cand[:1], w - len(cand))])
                u_all[b, r : r + K_AUG, uo : uo + TILE] = qf[ids].T
                v_all[b, r : r + K_AUG, vo : vo + w] = cf[pad].T
                core_map.append((d, ids))
            else:
                core_map.append(None)
        maps.append(core_map)
    return widths, u_all, v_all, maps


# ------------------------------------------------------------- device build


def slot_layout(widths):
    """Packs of up to 4 equal-width slots, round-robin over 3 PE quadrants.

    Returns (packs, uq, vq): packs = [(s0, P, w, quadrant, uoff, voff)];
    uq/vq = per-quadrant column capacity (max across quadrants).
    """
    S = len(widths)
    raw = []
    s = 0
    while s < S:
        p = 1
        while p < 4 and s + p < S and widths[s + p] == widths[s]:
            p += 1
        raw.append((s, p, widths[s]))
        s += p
    # contiguous quadrant blocks: quadrant 0's packs run first (its DMA
    # lands first), then 1, then 2 — PE never waits on a late quadrant
    tot = sum(P * w for _, P, w in raw)
    packs = []
    ucols = [0, 0, 0]
    vcols = [0, 0, 0]
    g = 0
    for s0, P, w in raw:
        if vcols[g] >= (tot + 2) // 3 and g < 2:
            g += 1
        packs.append((s0, P, w, g, ucols[g], vcols[g]))
        ucols[g] += P * TILE
        vcols[g] += P * w
    return packs, max(ucols), max(vcols)


def build_knn_nc(widths, do_compile=True):
    S = len(widths)
    packs, uq, vq = slot_layout(widths)

    nc = bacc.Bacc("TRN2", target_bir_lowering=False, debug=False)
    u_d = nc.dram_tensor("u_in", [3 * K_AUG, uq], BF16, kind="ExternalInput").ap()
    v_d = nc.dram_tensor("v_in", [3 * K_AUG, vq], BF16, kind="ExternalInput").ap()
    out_d = nc.dram_tensor("out", [TILE, S], F32, kind="ExternalOutput").ap()

    with tile.TileContext(nc) as tc, ExitStack() as ctx:
        const_pool = ctx.enter_context(tc.tile_pool(name="const", bufs=1))
        psum_pool = ctx.enter_context(tc.tile_pool(name="psum", bufs=2, space="PSUM"))
        dd_pool = ctx.enter_context(tc.tile_pool(name="dd", bufs=3))
        f1_pool = ctx.enter_context(tc.tile_pool(name="f1", bufs=2))
        f2_pool = ctx.enter_context(tc.tile_pool(name="f2", bufs=2))

        # quadrant q data sits on SBUF partitions 32q..32q+15 so the three
        # input DMAs write 48 partitions concurrently (3x bandwidth); separate
        # tiles per quadrant give per-quadrant dependencies, and v halves let
        # the first packs start after ~half a quadrant transfer
        u_ts, v_ts = [], []
        for g in range(3):
            rows = 32 * g + K_AUG
            ut = const_pool.tile([rows, uq], BF16, tag=f"u{g}")
            vt = const_pool.tile([rows, vq], BF16, tag=f"v{g}")
            r0 = 32 * g
            nc.scalar.dma_start(ut[r0 : r0 + K_AUG, :], u_d[K_AUG * g : K_AUG * (g + 1), :])
            vh = vq // 2
            nc.sync.dma_start(vt[r0 : r0 + K_AUG, :vh], v_d[K_AUG * g : K_AUG * (g + 1), :vh])
            nc.sync.dma_start(vt[r0 : r0 + K_AUG, vh:], v_d[K_AUG * g : K_AUG * (g + 1), vh:])
            u_ts.append(ut)
            v_ts.append(vt)

        outs = const_pool.tile([TILE, S], F32, tag="outs")

        # greedy 3-engine balance (ns models; ACT 1.2GHz, DVE 0.96GHz, GPS)
        act_busy = dve_busy = gps_busy = 0.0
        half_s = packs[(len(packs) // 2)][0]
        q3_s = packs[(len(packs) * 7 // 8)][0]
        for pi, (s0, P, w, g, uoff, voff) in enumerate(packs):
            ps = psum_pool.tile([TILE, 4, WMAX], F32, tag="ps")
            for j in range(P):
                nc.tensor.matmul(
                    ps[:, j, :w],
                    u_ts[g][32 * g : 32 * g + K_AUG, uoff + j * TILE : uoff + (j + 1) * TILE],
                    v_ts[g][32 * g : 32 * g + K_AUG, voff + j * w : voff + (j + 1) * w],
                    start=True,
                    stop=True,
                )
            n = P * w
            act_copy = (352 + n) / 1.2
            dve_foldall = (174 + 0.625 * n) / 0.96  # fold/2 + fold/4 + reduce
            dve_foldtail = (116 + 0.375 * n) / 0.96  # fold/4 + reduce (GPS did fold1)
            gps_fold = 300 + 2.2 * n / 2
            dve_direct = (120 + n) / 0.96
            last = pi == len(packs) - 1
            cost = {
                "A": max(act_busy + act_copy, dve_busy + dve_foldall, gps_busy),
                "C": max(act_busy, dve_busy + dve_direct, gps_busy),
            }
            path = "C" if last else min(cost, key=cost.get)
            if path == "C":
                dve_busy += dve_direct
                nc.vector.tensor_reduce(
                    outs[:, s0 : s0 + P], ps[:, :P, :w], axis=mybir.AxisListType.X, op=MIN
                )
            else:
                act_busy += act_copy
                dd = dd_pool.tile([TILE, 4, WMAX], BF16, tag="dd")
                nc.scalar.copy(dd[:, :P, :w], ps[:, :P, :w])
                f1 = f1_pool.tile([TILE, 4, WMAX // 2], BF16, tag="f1")
                eng = nc.gpsimd if path == "B" else nc.vector
                if path == "B":
                    gps_busy += gps_fold
                else:
                    dve_busy += dve_foldall - dve_foldtail
                eng.tensor_tensor(
                    f1[:, :P, : w // 2], dd[:, :P, : w // 2], dd[:, :P, w // 2 : w], MIN
                )
                dve_busy += dve_foldtail
                f2 = f2_pool.tile([TILE, 4, WMAX // 4], BF16, tag="f2")
                nc.vector.tensor_tensor(
                    f2[:, :P, : w // 4], f1[:, :P, : w // 4], f1[:, :P, w // 4 : w // 2], MIN
                )
                nc.vector.tensor_reduce(
                    outs[:, s0 : s0 + P], f2[:, :P, : w // 4], axis=mybir.AxisListType.X, op=MIN
                )
            if s0 + P == half_s:
                nc.sync.dma_start(out_d[:, :half_s], outs[:, :half_s])
            elif s0 + P == q3_s:
                nc.sync.dma_start(out_d[:, half_s:q3_s], outs[:, half_s:q3_s])

        nc.sync.dma_start(out_d[:, q3_s:], outs[:, q3_s:])

    if do_compile:
        nc.compile()
    return nc


_NC_CACHE = {}


def _get_nc(widths):
    if widths not in _NC_CACHE:
        _NC_CACHE[widths] = build_knn_nc(widths)
    return _NC_CACHE[widths]


def run_device(widths, u_all, v_all, trace: bool = False):
    from concourse.bass_utils import run_bass_kernel_spmd

    nc = _get_nc(widths)
    in_maps = [{"u_in": u_all[c], "v_in": v_all[c]} for c in range(N_CORES)]
    res = run_bass_kernel_spmd(nc, in_maps, list(range(N_CORES)), trace=trace)
    return res


def postprocess(res, maps):
    """Combine slot row-minima into the two mean losses (fp64)."""
    sums = np.zeros(2, np.float64)
    counts = np.zeros(2, np.int64)
    for c in range(N_CORES):
        out = np.asarray(res.results[c]["out"], np.float64)  # [128, S]
        # split tiles: same (dir, ids) may appear in multiple slots
        seen = {}
        for s, m in enumerate(maps[c]):
            if m is None:
                continue
            d, ids = m
            key = (d, ids[0])
            if key in seen:
                seen[key] = np.minimum(seen[key], out[:, s])
            else:
                seen[key] = out[:, s]
        for (d, _), vals in seen.items():
            sums[d] += vals.sum()
            counts[d] += len(vals)
    assert counts[0] == B * N and counts[1] == B * M, (counts, B * N)
    return np.float32(sums[0] / counts[0]), np.float32(sums[1] / counts[1])


def kernel(source: np.ndarray, target: np.ndarray):
    widths, u_all, v_all, maps = prep(source, target)
    res = run_device(widths, u_all, v_all)
    return postprocess(res, maps)


# revision 19
# speedup vs baseline: 1.3522x; 1.0115x over previous
"""Chamfer distance kernel for Trainium2 (Bass/Tile), SPMD over 8 NeuronCores.

Problem: source [8, 4096, 3], target [8, 4096, 3] float32.
  distance[b, n, m] = sum_c (source[b,n,c] - target[b,m,c])^2
  loss_src = mean_n min_m distance ; loss_dst = mean_m min_n distance
  returns (loss_src, loss_dst)

Sharding: batch b -> core b (data parallel; final means on host).

Grid-pruned exact KNN (IVF-style):
  Host prep (no pairwise point distances — bin geometry only):
    * Each direction (src->dst queries, dst->src queries) is tiled into 32
      compact tiles of 128 query points via recursive median bisection.
    * Reference points are binned on a uniform grid (cell h). For every
      query q, r(q) = dist(q, nearest occupied bin center) + half-diagonal
      upper-bounds its NN distance; every bin with mindist(q, bin) <= r(q)
      may hold the NN. The per-tile candidate set is the union of selected
      bins' members — provably a superset of every query's NN, so the
      tile row-min over candidates is the exact NN distance.
    * Candidate lists are chunked to <=512, slots sorted by width (desc),
      and per-slot widths maximized across the 8 cores (SPMD: one program).

  Device (per core, S ~ 65 slots):
    * slot = one bf16 matmul [16,128]x[16,w] -> PSUM [128,w] fp32 using the
      split-precision K=16 factorization (hi/lo bf16 pairs make the fp32
      products near-exact; see make_factors), then a row-min:
        - ACT path: ScalarE copies PSUM->SBUF bf16; DVE tensor_tensor_reduce
          folds halves (min) and accumulates the row-min in one op.
        - DVE path: DVE tensor_reduce min straight from PSUM (fp32, 1x).
      Paths are assigned greedily at build time to balance ScalarE vs DVE.
    * Output [128, S] fp32 row-minima; host combines split-tile slots with
      np.minimum and takes the final means in fp64.
"""

import os
import sys

import numpy as np

_TRN_REPO = "/opt/trn_rl_repo"
if _TRN_REPO not in sys.path and os.path.isdir(_TRN_REPO):
    sys.path.insert(0, _TRN_REPO)

from contextlib import ExitStack

import ml_dtypes
from scipy.spatial import cKDTree

import concourse.bacc as bacc
import concourse.mybir as mybir
import concourse.tile as tile

F32 = mybir.dt.float32
BF16 = mybir.dt.bfloat16
MIN = mybir.AluOpType.min
BF16NP = ml_dtypes.bfloat16
MIN_INIT = 1e30
K_AUG = 16  # rows of the split-precision augmented factors

B, N, M, C = 8, 4096, 4096, 3
N_CORES = 8
TILE = 128
WMAX = 512  # max slot width (one PSUM bank of fp32)
GRID_H = 0.018


# ---------------------------------------------------------------- host prep


def _split_bf16(x):
    x = np.asarray(x, np.float32)
    hi = x.astype(BF16NP)
    lo = (x - hi.astype(np.float32)).astype(BF16NP)
    return hi, lo


def make_factors(pts):
    """Per-point factor rows so that d(q, c) = sum_k QF[q, k] * CF[c, k].

    QF (query role):     [qh*3, qh*3, ql*3, ql*3, ah, al, 1, 1]
    CF (candidate role): [-2ch*3, -2cl*3, -2ch*3, -2cl*3, 1, 1, bh, bl]
    with x = xh + xl bf16 splits and a = ||q||^2, b = ||c||^2 split hi/lo.
    """
    p = np.asarray(pts, np.float32)
    ph, pl = _split_bf16(p)  # [n, 3] each
    nrm = (p.astype(np.float64) ** 2).sum(-1)
    nh, nl = _split_bf16(nrm)
    n = len(p)
    qf = np.zeros((n, K_AUG), BF16NP)
    qf[:, 0:3] = ph
    qf[:, 3:6] = ph
    qf[:, 6:9] = pl
    qf[:, 9:12] = pl
    qf[:, 12] = nh
    qf[:, 13] = nl
    qf[:, 14] = 1.0
    qf[:, 15] = 1.0
    cf = np.zeros((n, K_AUG), BF16NP)
    m2h = (-2.0 * ph.astype(np.float32)).astype(BF16NP)
    m2l = (-2.0 * pl.astype(np.float32)).astype(BF16NP)
    cf[:, 0:3] = m2h
    cf[:, 3:6] = m2l
    cf[:, 6:9] = m2h
    cf[:, 9:12] = m2l
    cf[:, 12] = 1.0
    cf[:, 13] = 1.0
    cf[:, 14] = nh
    cf[:, 15] = nl
    return qf, cf


def bisect_tiles(pts, tsize=TILE):
    """Permutation grouping points into compact boxes of `tsize`."""
    out = []

    def rec(ids):
        if len(ids) <= tsize:
            out.append(ids)
            return
        p = pts[ids]
        d = int(np.argmax(p.max(0) - p.min(0)))
        k = (len(ids) // 2 // tsize) * tsize or tsize
        part = np.argpartition(p[:, d], k)
        rec(ids[part[:k]])
        rec(ids[part[k:]])

    rec(np.arange(len(pts)))
    return np.concatenate(out)


def tile_candidates(q, t, h):
    """Exact-NN-complete candidate target ids per 128-query tile.

    Bin-granular: only grid geometry is used (no point-point distances).
    Returns (list_of_q_id_arrays, list_of_cand_id_arrays).
    """
    q = q.astype(np.float64)
    t = t.astype(np.float64)
    lo = np.minimum(q.min(0), t.min(0)) - 1e-9
    tb = np.floor((t - lo) / h).astype(np.int64)
    keys, inv = np.unique(tb, axis=0, return_inverse=True)
    order_m = np.argsort(inv, kind="stable")
    bin_start = np.searchsorted(inv[order_m], np.arange(len(keys) + 1))
    centers = lo + (keys + 0.5) * h
    hd = h * np.sqrt(3) / 2
    tree = cKDTree(centers)
    bin_lo_all = lo + keys * h
    bin_hi_all = bin_lo_all + h
    kq = min(8, len(keys))
    _, ci = tree.query(q, k=kq)
    ci = ci.reshape(len(q), kq)
    far = np.maximum(
        np.abs(q[:, None, :] - bin_lo_all[ci]), np.abs(q[:, None, :] - bin_hi_all[ci])
    )
    r = np.sqrt((far**2).sum(-1).min(1))  # NN distance upper bound per query
    bin_lo = lo + keys * h
    bin_hi = bin_lo + h
    order = bisect_tiles(q)
    q_tiles, cand_tiles = [], []
    for ti in range(0, len(q), TILE):
        ids = order[ti : ti + TILE]
        s = q[ids]
        balls = tree.query_ball_point(s, r[ids] + hd)
        sel = np.zeros(len(keys), bool)
        for j, bl in enumerate(balls):
            bl = np.asarray(bl, dtype=np.int64)
            near = np.maximum(np.maximum(bin_lo[bl] - s[j], s[j] - bin_hi[bl]), 0)
            ok = (near**2).sum(-1) <= r[ids[j]] ** 2
            sel[bl[ok]] = True
        cand = np.concatenate(
            [order_m[bin_start[k] : bin_start[k + 1]] for k in np.nonzero(sel)[0]]
        )
        q_tiles.append(ids)
        cand_tiles.append(cand)
    return q_tiles, cand_tiles


def prep(source, target, h=GRID_H):
    """Build per-core slot tensors U [16, 128*S], V [16, sum(widths)].

    Returns (widths, u_all [B,16,128*S], v_all [B,16,Vtot], slot_maps) where
    slot_maps[core] = list of (direction, n_slots_for_tile) aligned with the
    tile traversal; real slots per core are the first len(map) entries after
    per-core sorting (we keep explicit per-core slot lists instead).
    """
    src = np.asarray(source, np.float32)
    tgt = np.asarray(target, np.float32)
    per_core = []  # core -> list of (dir, width_used, q_ids, cand_ids)
    for b in range(B):
        sf_q, sf_c = make_factors(src[b])
        tf_q, tf_c = make_factors(tgt[b])
        slots = []
        for d, (q, t, qf, cf) in enumerate(
            [
                (src[b], tgt[b], sf_q, tf_c),
                (tgt[b], src[b], tf_q, sf_c),
            ]
        ):
            q_tiles, cand_tiles = tile_candidates(q, t, h)
            for ids, cand in zip(q_tiles, cand_tiles):
                for c0 in range(0, len(cand), WMAX):
                    chunk = cand[c0 : c0 + WMAX]
                    slots.append((d, len(chunk), ids, chunk, qf, cf))
        slots.sort(key=lambda s: -s[1])
        per_core.append(slots)

    S = max(len(s) for s in per_core)
    widths = np.zeros(S, np.int64)
    for slots in per_core:
        for i, sl in enumerate(slots):
            widths[i] = max(widths[i], sl[1])
    widths = np.minimum((widths + 31) // 32 * 32, WMAX)
    widths = np.maximum(widths, 64)
    # pad each quad-pack to its max width so packs stay equal-width (one
    # ACT copy / DVE fold+reduce per pack) without blanket 128-rounding
    for i in range(0, S, 4):
        widths[i : i + 4] = widths[i]
    widths = tuple(int(w) for w in widths)

    packs, uq, vq = slot_layout(widths)
    slot_pos = {}
    for s0, P, w, g, uoff, voff in packs:
        for j in range(P):
            slot_pos[s0 + j] = (g, uoff + j * TILE, voff + j * w)

    u_all = np.zeros((B, 3 * K_AUG, uq), BF16NP)
    v_all = np.zeros((B, 3 * K_AUG, vq), BF16NP)
    maps = []
    for b, slots in enumerate(per_core):
        core_map = []
        for i in range(S):
            w = widths[i]
            g, uo, vo = slot_pos[i]
            r = K_AUG * g
            if i < len(slots):
                d, wu, ids, cand, qf, cf = slots[i]
                pad = np.concatenate([cand, np.repeat(cand[:1], w - len(cand))])
                u_all[b, r : r + K_AUG, uo : uo + TILE] = qf[ids].T
                v_all[b, r : r + K_AUG, vo : vo + w] = cf[pad].T
                core_map.append((d, ids))
            else:
                core_map.append(None)
        maps.append(core_map)
    return widths, u_all, v_all, maps


# ------------------------------------------------------------- device build


def slot_layout(widths):
    """Packs of up to 4 equal-width slots, round-robin over 3 PE quadrants.

    Returns (packs, uq, vq): packs = [(s0, P, w, quadrant, uoff, voff)];
    uq/vq = per-quadrant column capacity (max across quadrants).
    """
    S = len(widths)
    raw = []
    s = 0
    while s < S:
        p = 1
        while p < 4 and s + p < S and widths[s + p] == widths[s]:
            p += 1
        raw.append((s, p, widths[s]))
        s += p
    # contiguous quadrant blocks: quadrant 0's packs run first (its DMA
    # lands first), then 1, then 2 — PE never waits on a late quadrant
    tot = sum(P * w for _, P, w in raw)
    packs = []
    ucols = [0, 0, 0]
    vcols = [0, 0, 0]
    g = 0
    for s0, P, w in raw:
        if vcols[g] >= (tot + 2) // 3 and g < 2:
            g += 1
        packs.append((s0, P, w, g, ucols[g], vcols[g]))
        ucols[g] += P * TILE
        vcols[g] += P * w
    return packs, max(ucols), max(vcols)


def build_knn_nc(widths, do_compile=True):
    S = len(widths)
    packs, uq, vq = slot_layout(widths)

    nc = bacc.Bacc("TRN2", target_bir_lowering=False, debug=False)
    u_d = nc.dram_tensor("u_in", [3 * K_AUG, uq], BF16, kind="ExternalInput").ap()
    v_d = nc.dram_tensor("v_in", [3 * K_AUG, vq], BF16, kind="ExternalInput").ap()
    out_d = nc.dram_tensor("out", [TILE, S], F32, kind="ExternalOutput").ap()

    with tile.TileContext(nc) as tc, ExitStack() as ctx:
        const_pool = ctx.enter_context(tc.tile_pool(name="const", bufs=1))
        psum_pool = ctx.enter_context(tc.tile_pool(name="psum", bufs=2, space="PSUM"))
        dd_pool = ctx.enter_context(tc.tile_pool(name="dd", bufs=3))
        f1_pool = ctx.enter_context(tc.tile_pool(name="f1", bufs=2))
        f2_pool = ctx.enter_context(tc.tile_pool(name="f2", bufs=2))

        # quadrant q data sits on SBUF partitions 32q..32q+15 so the three
        # input DMAs write 48 partitions concurrently (3x bandwidth); separate
        # tiles per quadrant give per-quadrant dependencies, and v halves let
        # the first packs start after ~half a quadrant transfer
        u_ts, v_ts = [], []
        for g in range(3):
            rows = 32 * g + K_AUG
            ut = const_pool.tile([rows, uq], BF16, tag=f"u{g}")
            vt = const_pool.tile([rows, vq], BF16, tag=f"v{g}")
            r0 = 32 * g
            nc.scalar.dma_start(ut[r0 : r0 + K_AUG, :], u_d[K_AUG * g : K_AUG * (g + 1), :])
            vh = vq // 2
            nc.sync.dma_start(vt[r0 : r0 + K_AUG, :vh], v_d[K_AUG * g : K_AUG * (g + 1), :vh])
            nc.sync.dma_start(vt[r0 : r0 + K_AUG, vh:], v_d[K_AUG * g : K_AUG * (g + 1), vh:])
            u_ts.append(ut)
            v_ts.append(vt)

        outs = const_pool.tile([TILE, S], F32, tag="outs")

        # greedy 3-engine balance (ns models; ACT 1.2GHz, DVE 0.96GHz, GPS)
        act_busy = dve_busy = gps_busy = 0.0
        half_s = packs[(len(packs) // 2)][0]
        q3_s = packs[(len(packs) * 7 // 8)][0]
        for pi, (s0, P, w, g, uoff, voff) in enumerate(packs):
            ps = psum_pool.tile([TILE, 4, WMAX], F32, tag="ps")
            for j in range(P):
                nc.tensor.matmul(
                    ps[:, j, :w],
                    u_ts[g][32 * g : 32 * g + K_AUG, uoff + j * TILE : uoff + (j + 1) * TILE],
                    v_ts[g][32 * g : 32 * g + K_AUG, voff + j * w : voff + (j + 1) * w],
                    start=True,
                    stop=True,
                )
            n = P * w
            act_copy = (352 + n) / 1.2
            dve_foldall = (174 + 0.625 * n) / 0.96  # fold/2 + fold/4 + reduce
            dve_foldtail = (116 + 0.375 * n) / 0.96  # fold/4 + reduce (GPS did fold1)
            gps_fold = 300 + 2.2 * n / 2
            dve_direct = (120 + n) / 0.96
            last = pi == len(packs) - 1
            cost = {
                "A": max(act_busy + act_copy, dve_busy + dve_foldall, gps_busy),
                "C": max(act_busy, dve_busy + dve_direct, gps_busy),
            }
            path = "C" if last else min(cost, key=cost.get)
            if path == "C":
                dve_busy += dve_direct
                nc.vector.tensor_reduce(
                    outs[:, s0 : s0 + P], ps[:, :P, :w], axis=mybir.AxisListType.X, op=MIN
                )
            else:
                act_busy += act_copy
                dd = dd_pool.tile([TILE, 4, WMAX], BF16, tag="dd")
                nc.scalar.copy(dd[:, :P, :w], ps[:, :P, :w])
                f1 = f1_pool.tile([TILE, 4, WMAX // 2], BF16, tag="f1")
                eng = nc.gpsimd if path == "B" else nc.vector
                if path == "B":
                    gps_busy += gps_fold
                else:
                    dve_busy += dve_foldall - dve_foldtail
                eng.tensor_tensor(
                    f1[:, :P, : w // 2], dd[:, :P, : w // 2], dd[:, :P, w // 2 : w], MIN
                )
                dve_busy += dve_foldtail
                f2 = f2_pool.tile([TILE, 4, WMAX // 4], BF16, tag="f2")
                nc.vector.tensor_tensor(
                    f2[:, :P, : w // 4], f1[:, :P, : w // 4], f1[:, :P, w // 4 : w // 2], MIN
                )
                nc.vector.tensor_reduce(
                    outs[:, s0 : s0 + P], f2[:, :P, : w // 4], axis=mybir.AxisListType.X, op=MIN
                )
            if s0 + P == half_s:
                nc.sync.dma_start(out_d[:, :half_s], outs[:, :half_s])
            elif s0 + P == q3_s:
                nc.sync.dma_start(out_d[:, half_s:q3_s], outs[:, half_s:q3_s])

        nc.sync.dma_start(out_d[:, q3_s:], outs[:, q3_s:])

    if do_compile:
        nc.compile()
    return nc


_NC_CACHE = {}


def _get_nc(widths):
    if widths not in _NC_CACHE:
        _NC_CACHE[widths] = build_knn_nc(widths)
    return _NC_CACHE[widths]


def run_device(widths, u_all, v_all, trace: bool = False):
    from concourse.bass_utils import run_bass_kernel_spmd

    nc = _get_nc(widths)
    in_maps = [{"u_in": u_all[c], "v_in": v_all[c]} for c in range(N_CORES)]
    res = run_bass_kernel_spmd(nc, in_maps, list(range(N_CORES)), trace=trace)
    return res


def postprocess(res, maps):
    """Combine slot row-minima into the two mean losses (fp64)."""
    sums = np.zeros(2, np.float64)
    counts = np.zeros(2, np.int64)
    for c in range(N_CORES):
        out = np.asarray(res.results[c]["out"], np.float64)  # [128, S]
        # split tiles: same (dir, ids) may appear in multiple slots
        seen = {}
        for s, m in enumerate(maps[c]):
            if m is None:
                continue
            d, ids = m
            key = (d, ids[0])
            if key in seen:
                seen[key] = np.minimum(seen[key], out[:, s])
            else:
                seen[key] = out[:, s]
        for (d, _), vals in seen.items():
            sums[d] += vals.sum()
            counts[d] += len(vals)
    assert counts[0] == B * N and counts[1] == B * M, (counts, B * N)
    return np.float32(sums[0] / counts[0]), np.float32(sums[1] / counts[1])


def kernel(source: np.ndarray, target: np.ndarray):
    widths, u_all, v_all, maps = prep(source, target)
    res = run_device(widths, u_all, v_all)
    return postprocess(res, maps)


# revision 20
# speedup vs baseline: 1.3797x; 1.0203x over previous
"""Chamfer distance kernel for Trainium2 (Bass/Tile), SPMD over 8 NeuronCores.

Problem: source [8, 4096, 3], target [8, 4096, 3] float32.
  distance[b, n, m] = sum_c (source[b,n,c] - target[b,m,c])^2
  loss_src = mean_n min_m distance ; loss_dst = mean_m min_n distance
  returns (loss_src, loss_dst)

Sharding: batch b -> core b (data parallel; final means on host).

Grid-pruned exact KNN (IVF-style):
  Host prep (no pairwise point distances — bin geometry only):
    * Each direction (src->dst queries, dst->src queries) is tiled into 32
      compact tiles of 128 query points via recursive median bisection.
    * Reference points are binned on a uniform grid (cell h). For every
      query q, r(q) = dist(q, nearest occupied bin center) + half-diagonal
      upper-bounds its NN distance; every bin with mindist(q, bin) <= r(q)
      may hold the NN. The per-tile candidate set is the union of selected
      bins' members — provably a superset of every query's NN, so the
      tile row-min over candidates is the exact NN distance.
    * Candidate lists are chunked to <=512, slots sorted by width (desc),
      and per-slot widths maximized across the 8 cores (SPMD: one program).

  Device (per core, S ~ 65 slots):
    * slot = one bf16 matmul [16,128]x[16,w] -> PSUM [128,w] fp32 using the
      split-precision K=16 factorization (hi/lo bf16 pairs make the fp32
      products near-exact; see make_factors), then a row-min:
        - ACT path: ScalarE copies PSUM->SBUF bf16; DVE tensor_tensor_reduce
          folds halves (min) and accumulates the row-min in one op.
        - DVE path: DVE tensor_reduce min straight from PSUM (fp32, 1x).
      Paths are assigned greedily at build time to balance ScalarE vs DVE.
    * Output [128, S] fp32 row-minima; host combines split-tile slots with
      np.minimum and takes the final means in fp64.
"""

import os
import sys

import numpy as np

_TRN_REPO = "/opt/trn_rl_repo"
if _TRN_REPO not in sys.path and os.path.isdir(_TRN_REPO):
    sys.path.insert(0, _TRN_REPO)

from contextlib import ExitStack

import ml_dtypes
from scipy.spatial import cKDTree

import concourse.bacc as bacc
import concourse.mybir as mybir
import concourse.tile as tile

F32 = mybir.dt.float32
BF16 = mybir.dt.bfloat16
MIN = mybir.AluOpType.min
BF16NP = ml_dtypes.bfloat16
MIN_INIT = 1e30
K_AUG = 16  # rows of the split-precision augmented factors

B, N, M, C = 8, 4096, 4096, 3
N_CORES = 8
TILE = 128
WMAX = 512  # max slot width (one PSUM bank of fp32)
GRID_H = 0.018


# ---------------------------------------------------------------- host prep


def _split_bf16(x):
    x = np.asarray(x, np.float32)
    hi = x.astype(BF16NP)
    lo = (x - hi.astype(np.float32)).astype(BF16NP)
    return hi, lo


def make_factors(pts):
    """Per-point factor rows so that d(q, c) = sum_k QF[q, k] * CF[c, k].

    QF (query role):     [qh*3, qh*3, ql*3, ql*3, ah, al, 1, 1]
    CF (candidate role): [-2ch*3, -2cl*3, -2ch*3, -2cl*3, 1, 1, bh, bl]
    with x = xh + xl bf16 splits and a = ||q||^2, b = ||c||^2 split hi/lo.
    """
    p = np.asarray(pts, np.float32)
    ph, pl = _split_bf16(p)  # [n, 3] each
    nrm = (p.astype(np.float64) ** 2).sum(-1)
    nh, nl = _split_bf16(nrm)
    n = len(p)
    qf = np.zeros((n, K_AUG), BF16NP)
    qf[:, 0:3] = ph
    qf[:, 3:6] = ph
    qf[:, 6:9] = pl
    qf[:, 9:12] = pl
    qf[:, 12] = nh
    qf[:, 13] = nl
    qf[:, 14] = 1.0
    qf[:, 15] = 1.0
    cf = np.zeros((n, K_AUG), BF16NP)
    m2h = (-2.0 * ph.astype(np.float32)).astype(BF16NP)
    m2l = (-2.0 * pl.astype(np.float32)).astype(BF16NP)
    cf[:, 0:3] = m2h
    cf[:, 3:6] = m2l
    cf[:, 6:9] = m2h
    cf[:, 9:12] = m2l
    cf[:, 12] = 1.0
    cf[:, 13] = 1.0
    cf[:, 14] = nh
    cf[:, 15] = nl
    return qf, cf


def bisect_tiles(pts, tsize=TILE):
    """Permutation grouping points into compact boxes of `tsize`."""
    out = []

    def rec(ids):
        if len(ids) <= tsize:
            out.append(ids)
            return
        p = pts[ids]
        d = int(np.argmax(p.max(0) - p.min(0)))
        k = (len(ids) // 2 // tsize) * tsize or tsize
        part = np.argpartition(p[:, d], k)
        rec(ids[part[:k]])
        rec(ids[part[k:]])

    rec(np.arange(len(pts)))
    return np.concatenate(out)


def tile_candidates(q, t, h):
    """Exact-NN-complete candidate target ids per 128-query tile.

    Bin-granular: only grid geometry is used (no point-point distances).
    Returns (list_of_q_id_arrays, list_of_cand_id_arrays).
    """
    q = q.astype(np.float64)
    t = t.astype(np.float64)
    lo = np.minimum(q.min(0), t.min(0)) - 1e-9
    tb = np.floor((t - lo) / h).astype(np.int64)
    keys, inv = np.unique(tb, axis=0, return_inverse=True)
    order_m = np.argsort(inv, kind="stable")
    bin_start = np.searchsorted(inv[order_m], np.arange(len(keys) + 1))
    centers = lo + (keys + 0.5) * h
    hd = h * np.sqrt(3) / 2
    tree = cKDTree(centers)
    bin_lo_all = lo + keys * h
    bin_hi_all = bin_lo_all + h
    kq = min(8, len(keys))
    _, ci = tree.query(q, k=kq)
    ci = ci.reshape(len(q), kq)
    far = np.maximum(
        np.abs(q[:, None, :] - bin_lo_all[ci]), np.abs(q[:, None, :] - bin_hi_all[ci])
    )
    r = np.sqrt((far**2).sum(-1).min(1))  # NN distance upper bound per query
    bin_lo = lo + keys * h
    bin_hi = bin_lo + h
    order = bisect_tiles(q)
    q_tiles, cand_tiles = [], []
    for ti in range(0, len(q), TILE):
        ids = order[ti : ti + TILE]
        s = q[ids]
        balls = tree.query_ball_point(s, r[ids] + hd)
        sel = np.zeros(len(keys), bool)
        for j, bl in enumerate(balls):
            bl = np.asarray(bl, dtype=np.int64)
            near = np.maximum(np.maximum(bin_lo[bl] - s[j], s[j] - bin_hi[bl]), 0)
            ok = (near**2).sum(-1) <= r[ids[j]] ** 2
            sel[bl[ok]] = True
        cand = np.concatenate(
            [order_m[bin_start[k] : bin_start[k + 1]] for k in np.nonzero(sel)[0]]
        )
        q_tiles.append(ids)
        cand_tiles.append(cand)
    return q_tiles, cand_tiles


def prep(source, target, h=GRID_H):
    """Build per-core slot tensors U [16, 128*S], V [16, sum(widths)].

    Returns (widths, u_all [B,16,128*S], v_all [B,16,Vtot], slot_maps) where
    slot_maps[core] = list of (direction, n_slots_for_tile) aligned with the
    tile traversal; real slots per core are the first len(map) entries after
    per-core sorting (we keep explicit per-core slot lists instead).
    """
    src = np.asarray(source, np.float32)
    tgt = np.asarray(target, np.float32)
    per_core = []  # core -> list of (dir, width_used, q_ids, cand_ids)
    for b in range(B):
        sf_q, sf_c = make_factors(src[b])
        tf_q, tf_c = make_factors(tgt[b])
        slots = []
        for d, (q, t, qf, cf) in enumerate(
            [
                (src[b], tgt[b], sf_q, tf_c),
                (tgt[b], src[b], tf_q, sf_c),
            ]
        ):
            q_tiles, cand_tiles = tile_candidates(q, t, h)
            for ids, cand in zip(q_tiles, cand_tiles):
                for c0 in range(0, len(cand), WMAX):
                    chunk = cand[c0 : c0 + WMAX]
                    slots.append((d, len(chunk), ids, chunk, qf, cf))
        slots.sort(key=lambda s: -s[1])
        per_core.append(slots)

    S = max(len(s) for s in per_core)
    widths = np.zeros(S, np.int64)
    for slots in per_core:
        for i, sl in enumerate(slots):
            widths[i] = max(widths[i], sl[1])
    widths = np.minimum((widths + 31) // 32 * 32, WMAX)
    widths = np.maximum(widths, 64)
    # pad each quad-pack to its max width so packs stay equal-width (one
    # ACT copy / DVE fold+reduce per pack) without blanket 128-rounding
    for i in range(0, S, 4):
        widths[i : i + 4] = widths[i]
    widths = tuple(int(w) for w in widths)

    packs, uq, vq = slot_layout(widths)
    slot_pos = {}
    for s0, P, w, g, uoff, voff in packs:
        for j in range(P):
            slot_pos[s0 + j] = (g, uoff + j * TILE, voff + j * w)

    u_all = np.zeros((B, 3 * K_AUG, uq), BF16NP)
    v_all = np.zeros((B, 3 * K_AUG, vq), BF16NP)
    maps = []
    for b, slots in enumerate(per_core):
        core_map = []
        for i in range(S):
            w = widths[i]
            g, uo, vo = slot_pos[i]
            r = K_AUG * g
            if i < len(slots):
                d, wu, ids, cand, qf, cf = slots[i]
                pad = np.concatenate([cand, np.repeat(cand[:1], w - len(cand))])
                u_all[b, r : r + K_AUG, uo : uo + TILE] = qf[ids].T
                v_all[b, r : r + K_AUG, vo : vo + w] = cf[pad].T
                core_map.append((d, ids))
            else:
                core_map.append(None)
        maps.append(core_map)
    return widths, u_all, v_all, maps


# ------------------------------------------------------------- device build


def slot_layout(widths):
    """Packs of up to 4 equal-width slots, round-robin over 3 PE quadrants.

    Returns (packs, uq, vq): packs = [(s0, P, w, quadrant, uoff, voff)];
    uq/vq = per-quadrant column capacity (max across quadrants).
    """
    S = len(widths)
    raw = []
    s = 0
    while s < S:
        p = 1
        while p < 4 and s + p < S and widths[s + p] == widths[s]:
            p += 1
        raw.append((s, p, widths[s]))
        s += p
    # contiguous quadrant blocks: quadrant 0's packs run first (its DMA
    # lands first), then 1, then 2 — PE never waits on a late quadrant
    tot = sum(P * w for _, P, w in raw)
    packs = []
    ucols = [0, 0, 0]
    vcols = [0, 0, 0]
    g = 0
    for s0, P, w in raw:
        if vcols[g] >= (tot + 2) // 3 and g < 2:
            g += 1
        packs.append((s0, P, w, g, ucols[g], vcols[g]))
        ucols[g] += P * TILE
        vcols[g] += P * w
    return packs, max(ucols), max(vcols)


def build_knn_nc(widths, do_compile=True):
    S = len(widths)
    packs, uq, vq = slot_layout(widths)

    nc = bacc.Bacc("TRN2", target_bir_lowering=False, debug=False)
    u_d = nc.dram_tensor("u_in", [3 * K_AUG, uq], BF16, kind="ExternalInput").ap()
    v_d = nc.dram_tensor("v_in", [3 * K_AUG, vq], BF16, kind="ExternalInput").ap()
    out_d = nc.dram_tensor("out", [TILE, S], F32, kind="ExternalOutput").ap()

    with tile.TileContext(nc) as tc, ExitStack() as ctx:
        const_pool = ctx.enter_context(tc.tile_pool(name="const", bufs=1))
        psum_pool = ctx.enter_context(tc.tile_pool(name="psum", bufs=2, space="PSUM"))
        dd_pool = ctx.enter_context(tc.tile_pool(name="dd", bufs=3))
        f1_pool = ctx.enter_context(tc.tile_pool(name="f1", bufs=2))
        f2_pool = ctx.enter_context(tc.tile_pool(name="f2", bufs=2))

        # quadrant q data sits on SBUF partitions 32q..32q+15 so input DMAs
        # write 48 partitions concurrently; each quadrant's u/v further split
        # into two half-tiles at a pack boundary so the first packs only wait
        # for the first half transfers
        qpacks = [[p for p in packs if p[3] == g] for g in range(3)]
        bounds = []  # per quadrant: (ub, vb) = cols in half 0
        for g in range(3):
            tot_v = sum(P * w for _, P, w, _, _, _ in qpacks[g])
            ub = vb = None
            for s0, P, w, _, uo, vo in qpacks[g]:
                if vo >= tot_v // 2:
                    ub, vb = uo, vo
                    break
            if ub is None:
                ub = sum(P * TILE for _, P, _, _, _, _ in qpacks[g])
                vb = tot_v
            bounds.append((ub, vb))
        u_ts, v_ts = [], []
        for g in range(3):
            rows = 32 * g + K_AUG
            r0 = 32 * g
            ub, vb = bounds[g]
            uta = const_pool.tile([rows, max(ub, 1)], BF16, tag=f"ua{g}")
            utb = const_pool.tile([rows, max(uq - ub, 1)], BF16, tag=f"ub{g}")
            vta = const_pool.tile([rows, max(vb, 1)], BF16, tag=f"va{g}")
            vtb = const_pool.tile([rows, max(vq - vb, 1)], BF16, tag=f"vb{g}")
            if ub:
                nc.scalar.dma_start(uta[r0 : r0 + K_AUG, :ub], u_d[K_AUG * g : K_AUG * (g + 1), :ub])
            if vb:
                nc.sync.dma_start(vta[r0 : r0 + K_AUG, :vb], v_d[K_AUG * g : K_AUG * (g + 1), :vb])
            if uq - ub:
                nc.scalar.dma_start(utb[r0 : r0 + K_AUG, : uq - ub], u_d[K_AUG * g : K_AUG * (g + 1), ub:])
            if vq - vb:
                nc.sync.dma_start(vtb[r0 : r0 + K_AUG, : vq - vb], v_d[K_AUG * g : K_AUG * (g + 1), vb:])
            u_ts.append((uta, utb))
            v_ts.append((vta, vtb))

        outs = const_pool.tile([TILE, S], F32, tag="outs")

        # greedy 3-engine balance (ns models; ACT 1.2GHz, DVE 0.96GHz, GPS)
        act_busy = dve_busy = gps_busy = 0.0
        half_s = packs[(len(packs) // 2)][0]
        q3_s = packs[(len(packs) * 7 // 8)][0]
        for pi, (s0, P, w, g, uoff, voff) in enumerate(packs):
            ps = psum_pool.tile([TILE, 4, WMAX], F32, tag="ps")
            ub, vb = bounds[g]
            half = voff >= vb
            utile = u_ts[g][half]
            vtile = v_ts[g][half]
            uo = uoff - (ub if half else 0)
            vo = voff - (vb if half else 0)
            for j in range(P):
                nc.tensor.matmul(
                    ps[:, j, :w],
                    utile[32 * g : 32 * g + K_AUG, uo + j * TILE : uo + (j + 1) * TILE],
                    vtile[32 * g : 32 * g + K_AUG, vo + j * w : vo + (j + 1) * w],
                    start=True,
                    stop=True,
                )
            n = P * w
            act_copy = (352 + n) / 1.2
            dve_foldall = (174 + 0.625 * n) / 0.96  # fold/2 + fold/4 + reduce
            dve_foldtail = (116 + 0.375 * n) / 0.96  # fold/4 + reduce (GPS did fold1)
            gps_fold = 300 + 2.2 * n / 2
            dve_direct = (120 + n) / 0.96
            last = pi == len(packs) - 1
            cost = {
                "A": max(act_busy + act_copy, dve_busy + dve_foldall, gps_busy),
                "C": max(act_busy, dve_busy + dve_direct, gps_busy),
            }
            path = "C" if last else min(cost, key=cost.get)
            if path == "C":
                dve_busy += dve_direct
                nc.vector.tensor_reduce(
                    outs[:, s0 : s0 + P], ps[:, :P, :w], axis=mybir.AxisListType.X, op=MIN
                )
            else:
                act_busy += act_copy
                dd = dd_pool.tile([TILE, 4, WMAX], BF16, tag="dd")
                nc.scalar.copy(dd[:, :P, :w], ps[:, :P, :w])
                f1 = f1_pool.tile([TILE, 4, WMAX // 2], BF16, tag="f1")
                eng = nc.gpsimd if path == "B" else nc.vector
                if path == "B":
                    gps_busy += gps_fold
                else:
                    dve_busy += dve_foldall - dve_foldtail
                eng.tensor_tensor(
                    f1[:, :P, : w // 2], dd[:, :P, : w // 2], dd[:, :P, w // 2 : w], MIN
                )
                dve_busy += dve_foldtail
                f2 = f2_pool.tile([TILE, 4, WMAX // 4], BF16, tag="f2")
                nc.vector.tensor_tensor(
                    f2[:, :P, : w // 4], f1[:, :P, : w // 4], f1[:, :P, w // 4 : w // 2], MIN
                )
                nc.vector.tensor_reduce(
                    outs[:, s0 : s0 + P], f2[:, :P, : w // 4], axis=mybir.AxisListType.X, op=MIN
                )
            if s0 + P == half_s:
                nc.sync.dma_start(out_d[:, :half_s], outs[:, :half_s])
            elif s0 + P == q3_s:
                nc.sync.dma_start(out_d[:, half_s:q3_s], outs[:, half_s:q3_s])

        nc.sync.dma_start(out_d[:, q3_s:], outs[:, q3_s:])

    if do_compile:
        nc.compile()
    return nc


_NC_CACHE = {}


def _get_nc(widths):
    if widths not in _NC_CACHE:
        _NC_CACHE[widths] = build_knn_nc(widths)
    return _NC_CACHE[widths]


def run_device(widths, u_all, v_all, trace: bool = False):
    from concourse.bass_utils import run_bass_kernel_spmd

    nc = _get_nc(widths)
    in_maps = [{"u_in": u_all[c], "v_in": v_all[c]} for c in range(N_CORES)]
    res = run_bass_kernel_spmd(nc, in_maps, list(range(N_CORES)), trace=trace)
    return res


def postprocess(res, maps):
    """Combine slot row-minima into the two mean losses (fp64)."""
    sums = np.zeros(2, np.float64)
    counts = np.zeros(2, np.int64)
    for c in range(N_CORES):
        out = np.asarray(res.results[c]["out"], np.float64)  # [128, S]
        # split tiles: same (dir, ids) may appear in multiple slots
        seen = {}
        for s, m in enumerate(maps[c]):
            if m is None:
                continue
            d, ids = m
            key = (d, ids[0])
            if key in seen:
                seen[key] = np.minimum(seen[key], out[:, s])
            else:
                seen[key] = out[:, s]
        for (d, _), vals in seen.items():
            sums[d] += vals.sum()
            counts[d] += len(vals)
    assert counts[0] == B * N and counts[1] == B * M, (counts, B * N)
    return np.float32(sums[0] / counts[0]), np.float32(sums[1] / counts[1])


def kernel(source: np.ndarray, target: np.ndarray):
    widths, u_all, v_all, maps = prep(source, target)
    res = run_device(widths, u_all, v_all)
    return postprocess(res, maps)


# revision 21
# speedup vs baseline: 1.3947x; 1.0109x over previous
"""Chamfer distance kernel for Trainium2 (Bass/Tile), SPMD over 8 NeuronCores.

Problem: source [8, 4096, 3], target [8, 4096, 3] float32.
  distance[b, n, m] = sum_c (source[b,n,c] - target[b,m,c])^2
  loss_src = mean_n min_m distance ; loss_dst = mean_m min_n distance
  returns (loss_src, loss_dst)

Sharding: batch b -> core b (data parallel; final means on host).

Grid-pruned exact KNN (IVF-style):
  Host prep (no pairwise point distances — bin geometry only):
    * Each direction (src->dst queries, dst->src queries) is tiled into 32
      compact tiles of 128 query points via recursive median bisection.
    * Reference points are binned on a uniform grid (cell h). For every
      query q, r(q) = dist(q, nearest occupied bin center) + half-diagonal
      upper-bounds its NN distance; every bin with mindist(q, bin) <= r(q)
      may hold the NN. The per-tile candidate set is the union of selected
      bins' members — provably a superset of every query's NN, so the
      tile row-min over candidates is the exact NN distance.
    * Candidate lists are chunked to <=512, slots sorted by width (desc),
      and per-slot widths maximized across the 8 cores (SPMD: one program).

  Device (per core, S = 64 slots, quad-packed, 3 PE quadrants):
    * slot = one bf16 matmul [16,128]x[16,w] -> PSUM [128,w] fp32 using the
      split-precision K=16 factorization (hi/lo bf16 pairs make the fp32
      products near-exact; see make_factors). Up to 4 equal-width slots
      share one 4-bank PSUM tile so one consumer op serves the whole pack.
    * Inputs live on PE quadrants (partitions 0/32/64, tile_position row
      tiling with K=16 -> 32-row tiles): three quadrant DMAs write 48
      partitions concurrently, each split into half-tiles at a pack
      boundary so the first matmul waits only for the first half. Packs
      are ordered quadrant 0 first (its DMA lands first).
    * Row-min per pack, greedily balanced between two paths:
        - ACT path: ScalarE copies PSUM->SBUF bf16; DVE TT-fold x2 (2x
          rate) then one tensor_reduce over the folded quarter.
        - DVE path: tensor_reduce min straight from PSUM (fp32, 1x).
    * Output [128, S] fp32 row-minima, DMA'd out in 3 chunks as slots
      complete; host min-combines split-tile slots and takes fp64 means.
"""

import os
import sys

import numpy as np

_TRN_REPO = "/opt/trn_rl_repo"
if _TRN_REPO not in sys.path and os.path.isdir(_TRN_REPO):
    sys.path.insert(0, _TRN_REPO)

from contextlib import ExitStack

import ml_dtypes
from scipy.spatial import cKDTree

import concourse.bacc as bacc
import concourse.mybir as mybir
import concourse.tile as tile

F32 = mybir.dt.float32
BF16 = mybir.dt.bfloat16
MIN = mybir.AluOpType.min
BF16NP = ml_dtypes.bfloat16
MIN_INIT = 1e30
K_AUG = 16  # rows of the split-precision augmented factors

B, N, M, C = 8, 4096, 4096, 3
N_CORES = 8
TILE = 128
WMAX = 512  # max slot width (one PSUM bank of fp32)
GRID_H = 0.018


# ---------------------------------------------------------------- host prep


def _split_bf16(x):
    x = np.asarray(x, np.float32)
    hi = x.astype(BF16NP)
    lo = (x - hi.astype(np.float32)).astype(BF16NP)
    return hi, lo


def make_factors(pts):
    """Per-point factor rows so that d(q, c) = sum_k QF[q, k] * CF[c, k].

    QF (query role):     [qh*3, qh*3, ql*3, ql*3, ah, al, 1, 1]
    CF (candidate role): [-2ch*3, -2cl*3, -2ch*3, -2cl*3, 1, 1, bh, bl]
    with x = xh + xl bf16 splits and a = ||q||^2, b = ||c||^2 split hi/lo.
    """
    p = np.asarray(pts, np.float32)
    ph, pl = _split_bf16(p)  # [n, 3] each
    nrm = (p.astype(np.float64) ** 2).sum(-1)
    nh, nl = _split_bf16(nrm)
    n = len(p)
    qf = np.zeros((n, K_AUG), BF16NP)
    qf[:, 0:3] = ph
    qf[:, 3:6] = ph
    qf[:, 6:9] = pl
    qf[:, 9:12] = pl
    qf[:, 12] = nh
    qf[:, 13] = nl
    qf[:, 14] = 1.0
    qf[:, 15] = 1.0
    cf = np.zeros((n, K_AUG), BF16NP)
    m2h = (-2.0 * ph.astype(np.float32)).astype(BF16NP)
    m2l = (-2.0 * pl.astype(np.float32)).astype(BF16NP)
    cf[:, 0:3] = m2h
    cf[:, 3:6] = m2l
    cf[:, 6:9] = m2h
    cf[:, 9:12] = m2l
    cf[:, 12] = 1.0
    cf[:, 13] = 1.0
    cf[:, 14] = nh
    cf[:, 15] = nl
    return qf, cf


def bisect_tiles(pts, tsize=TILE):
    """Permutation grouping points into compact boxes of `tsize`."""
    out = []

    def rec(ids):
        if len(ids) <= tsize:
            out.append(ids)
            return
        p = pts[ids]
        d = int(np.argmax(p.max(0) - p.min(0)))
        k = (len(ids) // 2 // tsize) * tsize or tsize
        part = np.argpartition(p[:, d], k)
        rec(ids[part[:k]])
        rec(ids[part[k:]])

    rec(np.arange(len(pts)))
    return np.concatenate(out)


def tile_candidates(q, t, h):
    """Exact-NN-complete candidate target ids per 128-query tile.

    Bin-granular: only grid geometry is used (no point-point distances).
    Returns (list_of_q_id_arrays, list_of_cand_id_arrays).
    """
    q = q.astype(np.float64)
    t = t.astype(np.float64)
    lo = np.minimum(q.min(0), t.min(0)) - 1e-9
    tb = np.floor((t - lo) / h).astype(np.int64)
    keys, inv = np.unique(tb, axis=0, return_inverse=True)
    order_m = np.argsort(inv, kind="stable")
    bin_start = np.searchsorted(inv[order_m], np.arange(len(keys) + 1))
    centers = lo + (keys + 0.5) * h
    hd = h * np.sqrt(3) / 2
    tree = cKDTree(centers)
    bin_lo_all = lo + keys * h
    bin_hi_all = bin_lo_all + h
    kq = min(8, len(keys))
    _, ci = tree.query(q, k=kq)
    ci = ci.reshape(len(q), kq)
    far = np.maximum(
        np.abs(q[:, None, :] - bin_lo_all[ci]), np.abs(q[:, None, :] - bin_hi_all[ci])
    )
    r = np.sqrt((far**2).sum(-1).min(1))  # NN distance upper bound per query
    bin_lo = lo + keys * h
    bin_hi = bin_lo + h
    order = bisect_tiles(q)
    q_tiles, cand_tiles = [], []
    for ti in range(0, len(q), TILE):
        ids = order[ti : ti + TILE]
        s = q[ids]
        balls = tree.query_ball_point(s, r[ids] + hd)
        sel = np.zeros(len(keys), bool)
        for j, bl in enumerate(balls):
            bl = np.asarray(bl, dtype=np.int64)
            near = np.maximum(np.maximum(bin_lo[bl] - s[j], s[j] - bin_hi[bl]), 0)
            ok = (near**2).sum(-1) <= r[ids[j]] ** 2
            sel[bl[ok]] = True
        cand = np.concatenate(
            [order_m[bin_start[k] : bin_start[k + 1]] for k in np.nonzero(sel)[0]]
        )
        q_tiles.append(ids)
        cand_tiles.append(cand)
    return q_tiles, cand_tiles


def prep(source, target, h=GRID_H):
    """Build per-core quadrant-packed factor tensors.

    Returns (widths, u_all [B,48,uq], v_all [B,48,vq], maps) where rows
    16g:16g+16 of u/v hold PE-quadrant g's data and maps[core][slot] =
    (direction, query_ids) or None for padding slots.
    """
    src = np.asarray(source, np.float32)
    tgt = np.asarray(target, np.float32)
    per_core = []  # core -> list of (dir, width_used, q_ids, cand_ids)
    for b in range(B):
        sf_q, sf_c = make_factors(src[b])
        tf_q, tf_c = make_factors(tgt[b])
        slots = []
        for d, (q, t, qf, cf) in enumerate(
            [
                (src[b], tgt[b], sf_q, tf_c),
                (tgt[b], src[b], tf_q, sf_c),
            ]
        ):
            q_tiles, cand_tiles = tile_candidates(q, t, h)
            for ids, cand in zip(q_tiles, cand_tiles):
                for c0 in range(0, len(cand), WMAX):
                    chunk = cand[c0 : c0 + WMAX]
                    slots.append((d, len(chunk), ids, chunk, qf, cf))
        slots.sort(key=lambda s: -s[1])
        per_core.append(slots)

    S = max(len(s) for s in per_core)
    widths = np.zeros(S, np.int64)
    for slots in per_core:
        for i, sl in enumerate(slots):
            widths[i] = max(widths[i], sl[1])
    widths = np.minimum((widths + 31) // 32 * 32, WMAX)
    widths = np.maximum(widths, 64)
    # pad each quad-pack to its max width so packs stay equal-width (one
    # ACT copy / DVE fold+reduce per pack) without blanket 128-rounding
    for i in range(0, S, 4):
        widths[i : i + 4] = widths[i]
    widths = tuple(int(w) for w in widths)

    packs, uq, vq = slot_layout(widths)
    slot_pos = {}
    for s0, P, w, g, uoff, voff in packs:
        for j in range(P):
            slot_pos[s0 + j] = (g, uoff + j * TILE, voff + j * w)

    u_all = np.zeros((B, 3 * K_AUG, uq), BF16NP)
    v_all = np.zeros((B, 3 * K_AUG, vq), BF16NP)
    maps = []
    for b, slots in enumerate(per_core):
        core_map = []
        for i in range(S):
            w = widths[i]
            g, uo, vo = slot_pos[i]
            r = K_AUG * g
            if i < len(slots):
                d, wu, ids, cand, qf, cf = slots[i]
                pad = np.concatenate([cand, np.repeat(cand[:1], w - len(cand))])
                u_all[b, r : r + K_AUG, uo : uo + TILE] = qf[ids].T
                v_all[b, r : r + K_AUG, vo : vo + w] = cf[pad].T
                core_map.append((d, ids))
            else:
                core_map.append(None)
        maps.append(core_map)
    return widths, u_all, v_all, maps


# ------------------------------------------------------------- device build


def slot_layout(widths):
    """Packs of up to 4 equal-width slots, round-robin over 3 PE quadrants.

    Returns (packs, uq, vq): packs = [(s0, P, w, quadrant, uoff, voff)];
    uq/vq = per-quadrant column capacity (max across quadrants).
    """
    S = len(widths)
    raw = []
    s = 0
    while s < S:
        p = 1
        while p < 4 and s + p < S and widths[s + p] == widths[s]:
            p += 1
        raw.append((s, p, widths[s]))
        s += p
    # contiguous quadrant blocks: quadrant 0's packs run first (its DMA
    # lands first), then 1, then 2 — PE never waits on a late quadrant
    tot = sum(P * w for _, P, w in raw)
    packs = []
    ucols = [0, 0, 0]
    vcols = [0, 0, 0]
    g = 0
    for s0, P, w in raw:
        if vcols[g] >= (tot + 2) // 3 and g < 2:
            g += 1
        packs.append((s0, P, w, g, ucols[g], vcols[g]))
        ucols[g] += P * TILE
        vcols[g] += P * w
    return packs, max(ucols), max(vcols)


def build_knn_nc(widths, do_compile=True):
    S = len(widths)
    packs, uq, vq = slot_layout(widths)

    nc = bacc.Bacc("TRN2", target_bir_lowering=False, debug=False)
    u_d = nc.dram_tensor("u_in", [3 * K_AUG, uq], BF16, kind="ExternalInput").ap()
    v_d = nc.dram_tensor("v_in", [3 * K_AUG, vq], BF16, kind="ExternalInput").ap()
    out_d = nc.dram_tensor("out", [TILE, S], F32, kind="ExternalOutput").ap()

    with tile.TileContext(nc) as tc, ExitStack() as ctx:
        const_pool = ctx.enter_context(tc.tile_pool(name="const", bufs=1))
        psum_pool = ctx.enter_context(tc.tile_pool(name="psum", bufs=2, space="PSUM"))
        dd_pool = ctx.enter_context(tc.tile_pool(name="dd", bufs=3))
        f1_pool = ctx.enter_context(tc.tile_pool(name="f1", bufs=2))
        f2_pool = ctx.enter_context(tc.tile_pool(name="f2", bufs=2))

        # quadrant q data sits on SBUF partitions 32q..32q+15 so input DMAs
        # write 48 partitions concurrently; each quadrant's u/v further split
        # into two half-tiles at a pack boundary so the first packs only wait
        # for the first half transfers
        qpacks = [[p for p in packs if p[3] == g] for g in range(3)]
        bounds = []  # per quadrant: (ub, vb) = cols in half 0
        for g in range(3):
            tot_v = sum(P * w for _, P, w, _, _, _ in qpacks[g])
            ub = vb = None
            for s0, P, w, _, uo, vo in qpacks[g]:
                if vo >= tot_v // 2:
                    ub, vb = uo, vo
                    break
            if ub is None:
                ub = sum(P * TILE for _, P, _, _, _, _ in qpacks[g])
                vb = tot_v
            bounds.append((ub, vb))
        u_ts, v_ts = [], []
        for g in range(3):
            rows = 32 * g + K_AUG
            r0 = 32 * g
            ub, vb = bounds[g]
            uta = const_pool.tile([rows, max(ub, 1)], BF16, tag=f"ua{g}")
            utb = const_pool.tile([rows, max(uq - ub, 1)], BF16, tag=f"ub{g}")
            vta = const_pool.tile([rows, max(vb, 1)], BF16, tag=f"va{g}")
            vtb = const_pool.tile([rows, max(vq - vb, 1)], BF16, tag=f"vb{g}")
            if ub:
                nc.scalar.dma_start(uta[r0 : r0 + K_AUG, :ub], u_d[K_AUG * g : K_AUG * (g + 1), :ub])
            if vb:
                nc.sync.dma_start(vta[r0 : r0 + K_AUG, :vb], v_d[K_AUG * g : K_AUG * (g + 1), :vb])
            if uq - ub:
                nc.scalar.dma_start(utb[r0 : r0 + K_AUG, : uq - ub], u_d[K_AUG * g : K_AUG * (g + 1), ub:])
            if vq - vb:
                nc.sync.dma_start(vtb[r0 : r0 + K_AUG, : vq - vb], v_d[K_AUG * g : K_AUG * (g + 1), vb:])
            u_ts.append((uta, utb))
            v_ts.append((vta, vtb))

        outs = const_pool.tile([TILE, S], F32, tag="outs")

        # greedy 3-engine balance (ns models; ACT 1.2GHz, DVE 0.96GHz, GPS)
        act_busy = dve_busy = gps_busy = 0.0
        half_s = packs[(len(packs) // 2)][0]
        q3_s = packs[(len(packs) * 7 // 8)][0]
        for pi, (s0, P, w, g, uoff, voff) in enumerate(packs):
            ps = psum_pool.tile([TILE, 4, WMAX], F32, tag="ps")
            ub, vb = bounds[g]
            half = voff >= vb
            utile = u_ts[g][half]
            vtile = v_ts[g][half]
            uo = uoff - (ub if half else 0)
            vo = voff - (vb if half else 0)
            for j in range(P):
                nc.tensor.matmul(
                    ps[:, j, :w],
                    utile[32 * g : 32 * g + K_AUG, uo + j * TILE : uo + (j + 1) * TILE],
                    vtile[32 * g : 32 * g + K_AUG, vo + j * w : vo + (j + 1) * w],
                    start=True,
                    stop=True,
                )
            n = P * w
            act_copy = (352 + n) / 1.2
            dve_foldall = (174 + 0.625 * n) / 0.96  # fold/2 + fold/4 + reduce
            dve_foldtail = (116 + 0.375 * n) / 0.96  # fold/4 + reduce (GPS did fold1)
            gps_fold = 300 + 2.2 * n / 2
            dve_direct = (120 + n) / 0.96
            last = pi == len(packs) - 1
            cost = {
                "A": max(act_busy + act_copy, dve_busy + dve_foldall, gps_busy),
                "C": max(act_busy, dve_busy + dve_direct, gps_busy),
            }
            path = "C" if last else min(cost, key=cost.get)
            if path == "C":
                dve_busy += dve_direct
                nc.vector.tensor_reduce(
                    outs[:, s0 : s0 + P], ps[:, :P, :w], axis=mybir.AxisListType.X, op=MIN
                )
            else:
                act_busy += act_copy
                dd = dd_pool.tile([TILE, 4, WMAX], BF16, tag="dd")
                nc.scalar.copy(dd[:, :P, :w], ps[:, :P, :w])
                f1 = f1_pool.tile([TILE, 4, WMAX // 2], BF16, tag="f1")
                eng = nc.gpsimd if path == "B" else nc.vector
                if path == "B":
                    gps_busy += gps_fold
                else:
                    dve_busy += dve_foldall - dve_foldtail
                eng.tensor_tensor(
                    f1[:, :P, : w // 2], dd[:, :P, : w // 2], dd[:, :P, w // 2 : w], MIN
                )
                dve_busy += dve_foldtail
                f2 = f2_pool.tile([TILE, 4, WMAX // 4], BF16, tag="f2")
                nc.vector.tensor_tensor(
                    f2[:, :P, : w // 4], f1[:, :P, : w // 4], f1[:, :P, w // 4 : w // 2], MIN
                )
                nc.vector.tensor_reduce(
                    outs[:, s0 : s0 + P], f2[:, :P, : w // 4], axis=mybir.AxisListType.X, op=MIN
                )
            if s0 + P == half_s:
                nc.sync.dma_start(out_d[:, :half_s], outs[:, :half_s])
            elif s0 + P == q3_s:
                nc.sync.dma_start(out_d[:, half_s:q3_s], outs[:, half_s:q3_s])

        nc.sync.dma_start(out_d[:, q3_s:], outs[:, q3_s:])

    if do_compile:
        nc.compile()
    return nc


_NC_CACHE = {}


def _get_nc(widths):
    if widths not in _NC_CACHE:
        _NC_CACHE[widths] = build_knn_nc(widths)
    return _NC_CACHE[widths]


def run_device(widths, u_all, v_all, trace: bool = False):
    from concourse.bass_utils import run_bass_kernel_spmd

    nc = _get_nc(widths)
    in_maps = [{"u_in": u_all[c], "v_in": v_all[c]} for c in range(N_CORES)]
    res = run_bass_kernel_spmd(nc, in_maps, list(range(N_CORES)), trace=trace)
    return res


def postprocess(res, maps):
    """Combine slot row-minima into the two mean losses (fp64)."""
    sums = np.zeros(2, np.float64)
    counts = np.zeros(2, np.int64)
    for c in range(N_CORES):
        out = np.asarray(res.results[c]["out"], np.float64)  # [128, S]
        # split tiles: same (dir, ids) may appear in multiple slots
        seen = {}
        for s, m in enumerate(maps[c]):
            if m is None:
                continue
            d, ids = m
            key = (d, ids[0])
            if key in seen:
                seen[key] = np.minimum(seen[key], out[:, s])
            else:
                seen[key] = out[:, s]
        for (d, _), vals in seen.items():
            sums[d] += vals.sum()
            counts[d] += len(vals)
    assert counts[0] == B * N and counts[1] == B * M, (counts, B * N)
    return np.float32(sums[0] / counts[0]), np.float32(sums[1] / counts[1])


def kernel(source: np.ndarray, target: np.ndarray):
    widths, u_all, v_all, maps = prep(source, target)
    res = run_device(widths, u_all, v_all)
    return postprocess(res, maps)
